# Initial kernel scaffold
#
"""Multi-head attention TRN2 kernel (B=2, N=2048, D=1024, H=16).

Sharding: tensor-parallel over heads. Each of the 8 cores owns 2 heads
(both batch elements) end-to-end through QKV projection and attention,
then the per-head attention outputs are AllGathered (per batch element)
and each core computes a 128-column slice of the output projection.

All matmuls run in float32r (TF32-like fp32 @ full PE rate, ~1.5e-4
scaled error). Softmax runs without max-subtraction (scores are O(5)
here; exp is safe in fp32): S^T is computed directly in key-major
layout via matmul(lhsT=kT, rhs=qT), exp'd elementwise on ScalarE, and
the softmax denominator comes from a ones-column appended to V in the
P^T@V matmul — no cross-partition reductions anywhere.

All pools are flat (top-level) with PSUM banks time-multiplexed by tag,
so the scheduler can interleave QKV, attention, collectives, and the
output projection across engines.

Self-contained: hardcodes shapes from the problem spec.
"""

import sys

for _p in ("/opt/trn_rl_repo", "/root/.axon_site/_ro/trn_rl_repo"):
    if _p not in sys.path:
        sys.path.append(_p)

import numpy as np
from contextlib import ExitStack

import concourse.bass as bass
import concourse.tile as tile
from concourse import mybir, bacc
from concourse.bass_utils import run_bass_kernel_spmd

F32 = mybir.dt.float32
F32R = mybir.dt.float32r
BF16 = mybir.dt.bfloat16
EXP = mybir.ActivationFunctionType.Exp

B = 2
N = 2048
D = 1024
H = 16
DEPTH = 64
TOK = B * N            # 4096 tokens total (both batches)
KC = D // 128          # 8 contraction chunks of 128
NBLK = TOK // 512      # 8 token blocks for streaming projections
SCALE = 1.0 / np.sqrt(DEPTH)
NCORES = 8
IBLK = 1024            # query-block width in attention
NSUB = IBLK // 512     # matmuls per psum tile (N<=512 for 4-byte dtypes)


def build_nc(reps=1, with_collective=True, qkv_dt=F32R, attn_dt=F32R,
             proj_dt=F32R):
    """Build the per-core kernel program.

    reps>1 wraps the compute in a For_i hardware loop for benchmarking
    (collectives are skipped: they cannot appear inside control flow).

    qkv_dt: dtype of x^T and QKV weights (the QKV matmuls).
    attn_dt: dtype of q^T/k^T/V2/P^T (the S^T and P^T@V matmuls).
    proj_dt: dtype of the AllGathered A^T and Wproj (projection matmuls).
    f32r ~1.5e-4 scaled err @2cyc/row; bf16 ~2e-3 @1cyc/row.
    """
    bench = reps > 1
    nc = bacc.Bacc(None)

    def dram_dt(dt):
        return F32 if dt == F32R else dt

    def cast(ap, dt):
        return ap.bitcast(F32R) if dt == F32R else ap

    xt = nc.dram_tensor("xt", [D, TOK], dram_dt(qkv_dt), kind="ExternalInput")
    wq = nc.dram_tensor("wq", [D, 128], dram_dt(qkv_dt), kind="ExternalInput")
    wk = nc.dram_tensor("wk", [D, 128], dram_dt(qkv_dt), kind="ExternalInput")
    wv = nc.dram_tensor("wv", [D, 128], dram_dt(qkv_dt), kind="ExternalInput")
    wp = nc.dram_tensor("wp", [D, 128], dram_dt(proj_dt), kind="ExternalInput")
    bq = nc.dram_tensor("bq", [128, 1], F32, kind="ExternalInput")
    bk = nc.dram_tensor("bk", [128, 1], F32, kind="ExternalInput")
    bv = nc.dram_tensor("bv", [128, 1], F32, kind="ExternalInput")
    bp = nc.dram_tensor("bp", [128, 1], F32, kind="ExternalInput")
    ident = nc.dram_tensor(
        "ident", [128, 128], dram_dt(attn_dt), kind="ExternalInput"
    )
    ones = nc.dram_tensor("ones", [128, 1], dram_dt(attn_dt), kind="ExternalInput")
    out = nc.dram_tensor("o", [128, TOK], F32, kind="ExternalOutput")

    # Collective staging (split per batch element so the b=0 AllGather
    # overlaps b=1 attention).
    HALF = N // 2
    ag_in = [
        [nc.dram_tensor(f"ag_in{b}_{h}", [128, HALF], proj_dt) for h in range(2)]
        for b in range(B)
    ]
    ag_out = [
        [
            nc.dram_tensor(f"ag_out{b}_{h}", [D, HALF], proj_dt,
                           addr_space="Shared")
            for h in range(2)
        ]
        for b in range(B)
    ]

    xt_r = xt.rearrange("(kc p) t -> p kc t", p=128)

    with tile.TileContext(nc) as tc, ExitStack() as ctx:
        wpool = ctx.enter_context(tc.tile_pool(name="w", bufs=1))
        qkpool = ctx.enter_context(tc.tile_pool(name="qk", bufs=1))
        vpool = ctx.enter_context(tc.tile_pool(name="v2", bufs=1))
        xpool = ctx.enter_context(tc.tile_pool(name="x", bufs=3))
        vtpool = ctx.enter_context(tc.tile_pool(name="vt", bufs=2))
        ptpool = ctx.enter_context(tc.tile_pool(name="pt", bufs=3))
        unpool = ctx.enter_context(tc.tile_pool(name="un", bufs=3))
        rpool = ctx.enter_context(tc.tile_pool(name="r", bufs=1))
        rdpool = ctx.enter_context(tc.tile_pool(name="rd", bufs=2, space="DRAM"))
        apool = ctx.enter_context(tc.tile_pool(name="ap", bufs=2))
        oupool = ctx.enter_context(tc.tile_pool(name="ou", bufs=2))
        # (SBUF/partition budget: w 17 + qk 32 + v2 16.6 + x 48 + vt 4 +
        #  pt 12 + un 24 + r 16 + ap 32 + ou 4 ~= 205 KB of 208)
        # PSUM budget (8 banks of 2KB/partition):
        #   ps_one (bufs=1): po [65,1024] (2x1)                   -> 2 banks
        #   ps_mm (bufs=2): mmA [128,512] (1x2) shared by qkv-proj groups,
        #                   v-transposes and out-proj groups      -> 2 banks
        #   ps_two (bufs=2): ss [128,1024] (2x2)                  -> 4 banks
        ps_one = ctx.enter_context(tc.tile_pool(name="ps1", bufs=1, space="PSUM"))
        ps_mm = ctx.enter_context(tc.tile_pool(name="psm", bufs=2, space="PSUM"))
        ps_two = ctx.enter_context(tc.tile_pool(name="ps2", bufs=2, space="PSUM"))

        # ---- weights / constants (loaded once, outside any rep loop) ----
        w_q = wpool.tile([128, KC, 128], qkv_dt, tag="w_q")
        w_k = wpool.tile([128, KC, 128], qkv_dt, tag="w_k")
        w_v = wpool.tile([128, KC, 128], qkv_dt, tag="w_v")
        w_p = wpool.tile([128, KC, 128], proj_dt, tag="w_p")
        for t, src, dt in (
            (w_q, wq, qkv_dt), (w_k, wk, qkv_dt), (w_v, wv, qkv_dt),
            (w_p, wp, proj_dt),
        ):
            nc.sync.dma_start(
                out=t, in_=cast(src.rearrange("(kc p) m -> p kc m", p=128), dt)
            )
        b_q = wpool.tile([128, 1], F32, tag="b_q")
        b_k = wpool.tile([128, 1], F32, tag="b_k")
        b_v = wpool.tile([128, 1], F32, tag="b_v")
        b_p = wpool.tile([128, 1], F32, tag="b_p")
        for t, src in ((b_q, bq), (b_k, bk), (b_v, bv), (b_p, bp)):
            nc.gpsimd.dma_start(out=t, in_=src[:])
        id_t = wpool.tile([128, 128], attn_dt, tag="id_t")
        nc.gpsimd.dma_start(out=id_t, in_=cast(ident[:], attn_dt))

        # qT/kT: [feature 128 (= 2 heads x 64), token 4096]; head hl in rows
        # hl*64:(hl+1)*64 so both S^T operands share a partition base.
        qT = qkpool.tile([128, TOK], attn_dt, tag="qT")
        kT = qkpool.tile([128, TOK], attn_dt, tag="kT")
        # V2: [token part, 32 token-chunks, 130]: v_h0 | ones | v_h1 | ones
        V2 = vpool.tile([128, TOK // 128, 130], attn_dt, tag="V2")
        nc.gpsimd.dma_start(
            out=V2[:, :, 64:65],
            in_=cast(ones[:].to_broadcast((128, TOK // 128, 1)), attn_dt),
        )
        nc.gpsimd.dma_start(
            out=V2[:, :, 129:130],
            in_=cast(ones[:].to_broadcast((128, TOK // 128, 1)), attn_dt),
        )

        rep_ctx = ExitStack()
        if bench:
            rep_ctx.enter_context(
                tc.For_i(
                    0,
                    reps,
                    1,
                    hint_engines=(
                        mybir.EngineType.PE,
                        mybir.EngineType.Activation,
                        mybir.EngineType.DVE,
                        mybir.EngineType.SP,
                    ),
                )
            )

        # ================= phase A: QKV projections =================
        for blk in range(NBLK):
            t0 = blk * 512
            xb = xpool.tile([128, KC, 512], qkv_dt, tag="xb")
            eng = nc.sync if blk % 2 == 0 else nc.gpsimd
            eng.dma_start(out=xb, in_=cast(xt_r[:, :, t0 : t0 + 512], qkv_dt))
            for name, w_t, b_t in (
                ("q", w_q, b_q), ("k", w_k, b_k), ("v", w_v, b_v)
            ):
                ps = ps_mm.tile([128, 512], F32, tag="mmA")
                for kc in range(KC):
                    nc.tensor.matmul(
                        out=ps,
                        lhsT=w_t[:, kc, :],
                        rhs=xb[:, kc, :],
                        start=(kc == 0),
                        stop=(kc == KC - 1),
                    )
                if name == "q":
                    nc.vector.tensor_scalar_add(
                        out=qT[:, t0 : t0 + 512], in0=ps, scalar1=b_t
                    )
                elif name == "k":
                    nc.vector.tensor_scalar_add(
                        out=kT[:, t0 : t0 + 512], in0=ps, scalar1=b_t
                    )
                else:
                    vtmp = vtpool.tile([128, 512], attn_dt, tag="vtmp")
                    nc.vector.tensor_scalar_add(out=vtmp, in0=ps, scalar1=b_t)
                    # transpose 4x [128,128] -> V2 token chunks
                    for s in range(4):
                        ch = blk * 4 + s
                        ps_t = ps_mm.tile([128, 128], attn_dt, tag="mmA")
                        nc.tensor.transpose(
                            out=ps_t,
                            in_=vtmp[:, s * 128 : (s + 1) * 128],
                            identity=id_t,
                        )
                        nc.vector.tensor_copy(out=V2[:, ch, 0:64], in_=ps_t[:, 0:64])
                        nc.vector.tensor_copy(
                            out=V2[:, ch, 65:129], in_=ps_t[:, 64:128]
                        )

        # ===== phase B: attention per (batch, head) + AllGathers =====
        for b in range(B):
            for ib in range(N // IBLK):
                for hl in range(2):
                    hs = hl * 64
                    voff = hl * 65
                    i0 = b * N + ib * IBLK
                    il = ib * IBLK  # local (within-batch) offset
                    ps_o = ps_one.tile([65, IBLK], F32, tag="po")
                    NJC = N // 128

                    def s_step(jc):
                        j0 = b * N + jc * 128
                        ps_s = ps_two.tile([128, IBLK], F32, tag="ss")
                        for su in range(NSUB):
                            nc.tensor.matmul(
                                out=ps_s[:, su * 512 : (su + 1) * 512],
                                lhsT=kT[hs : hs + 64, j0 : j0 + 128],
                                rhs=qT[
                                    hs : hs + 64,
                                    i0 + su * 512 : i0 + (su + 1) * 512,
                                ],
                                start=True,
                                stop=True,
                            )
                        pt = ptpool.tile([128, IBLK], attn_dt, tag="pt")
                        nc.scalar.activation(
                            out=pt, in_=ps_s, func=EXP, scale=float(SCALE)
                        )
                        return pt

                    # software pipeline: keep S^T one step ahead of PV in
                    # the PE stream so PE never waits on ScalarE's exp
                    pts = s_step(0)
                    for jc in range(NJC):
                        pt_cur = pts
                        if jc + 1 < NJC:
                            pts = s_step(jc + 1)
                        for su in range(NSUB):
                            nc.tensor.matmul(
                                out=ps_o[:, su * 512 : (su + 1) * 512],
                                lhsT=V2[:, ((b * N + jc * 128) // 128), voff : voff + 65],
                                rhs=pt_cur[:, su * 512 : (su + 1) * 512],
                                start=(jc == 0),
                                stop=(jc == NJC - 1),
                            )
                    # early-evict from PSUM: unnormalized rows + reciprocal,
                    # then normalize via a DRAM broadcast round-trip and
                    # stream the chunk straight to AllGather staging
                    un = unpool.tile([64, IBLK], F32, tag="un")
                    nc.vector.tensor_copy(out=un, in_=ps_o[0:64, :])
                    rb = rpool.tile([128, IBLK], F32, tag="rb")
                    nc.vector.reciprocal(out=rb[64:65, :], in_=ps_o[64:65, :])
                    rd = rdpool.tile([1, IBLK], F32, tag="rd")
                    nc.gpsimd.dma_start(out=rd, in_=rb[64:65, :])
                    rr = rpool.tile([64, IBLK], F32, tag="rr")
                    nc.gpsimd.dma_start(out=rr, in_=rd.to_broadcast((64, IBLK)))
                    unr = unpool.tile([64, IBLK], proj_dt, tag="unr")
                    nc.vector.tensor_mul(out=unr, in0=un, in1=rr)
                    nc.gpsimd.dma_start(
                        out=ag_in[b][ib][hs : hs + 64, :], in_=unr
                    )
                if with_collective:
                    # both heads of (b, ib-quarter) staged: gather it
                    nc.gpsimd.collective_compute(
                        "AllGather",
                        mybir.AluOpType.bypass,
                        ins=[ag_in[b][ib][:]],
                        outs=[ag_out[b][ib][:]],
                        replica_groups=[list(range(NCORES))],
                    )

        # ======= phase D: output projection (128 columns/core) =======
        for b in range(B):
            for ib in range(N // 512):
                hf = ib // 2
                ag_r = ag_out[b][hf].rearrange("(kc p) t -> p kc t", p=128)
                i0 = (ib % 2) * 512
                ab = apool.tile([128, KC, 512], proj_dt, tag="ab")
                eng = nc.sync if ib % 2 == 0 else nc.gpsimd
                eng.dma_start(out=ab, in_=ag_r[:, :, i0 : i0 + 512])
                ps = ps_mm.tile([128, 512], F32, tag="mmA")
                for kc in range(KC):
                    nc.tensor.matmul(
                        out=ps,
                        lhsT=w_p[:, kc, :],
                        rhs=ab[:, kc, :],
                        start=(kc == 0),
                        stop=(kc == KC - 1),
                    )
                ot = oupool.tile([128, 512], F32, tag="ot")
                nc.vector.tensor_scalar_add(out=ot, in0=ps, scalar1=b_p)
                nc.sync.dma_start(
                    out=out[:, b * N + ib * 512 : b * N + (ib + 1) * 512],
                    in_=ot,
                )

        rep_ctx.close()

    nc.compile()
    return nc


def np_dt(dt):
    return mybir.dt.np(F32 if dt == F32R else dt)


def prep_in_maps(x, Wqkv, bqkv, Wproj, bproj, qkv_dt=F32R, attn_dt=F32R,
                 proj_dt=F32R):
    x = np.asarray(x, dtype=np.float32)
    Wqkv = np.asarray(Wqkv, dtype=np.float32)
    bqkv = np.asarray(bqkv, dtype=np.float32)
    Wproj = np.asarray(Wproj, dtype=np.float32)
    bproj = np.asarray(bproj, dtype=np.float32)

    xtn = np.ascontiguousarray(x.reshape(TOK, D).T).astype(np_dt(qkv_dt))
    identity = np.eye(128, dtype=np_dt(attn_dt))
    ones_col = np.ones((128, 1), dtype=np_dt(attn_dt))

    # AllGather output rows are rank-major: row c*128 + hl*64 + d holds
    # feature (2c+hl)*64 + d; permute Wproj's contraction rows to match.
    wp_row_perm = np.empty(D, dtype=np.int64)
    for cc in range(NCORES):
        for hlhl in range(2):
            rows = np.arange(64)
            wp_row_perm[cc * 128 + hlhl * 64 + rows] = (2 * cc + hlhl) * 64 + rows

    # qkv column index for (head h, depth d, which): h*192 + d*3 + which
    d_idx = np.arange(DEPTH)
    in_maps = []
    for c in range(NCORES):
        h0, h1 = 2 * c, 2 * c + 1
        qcols = np.concatenate([h0 * 192 + 3 * d_idx, h1 * 192 + 3 * d_idx])
        kcols = qcols + 1
        vcols = qcols + 2
        in_maps.append(
            {
                "xt": xtn,
                "wq": np.ascontiguousarray(Wqkv[:, qcols]).astype(np_dt(qkv_dt)),
                "wk": np.ascontiguousarray(Wqkv[:, kcols]).astype(np_dt(qkv_dt)),
                "wv": np.ascontiguousarray(Wqkv[:, vcols]).astype(np_dt(qkv_dt)),
                "wp": np.ascontiguousarray(
                    Wproj[wp_row_perm, 128 * c : 128 * (c + 1)]
                ).astype(np_dt(proj_dt)),
                "bq": np.ascontiguousarray(bqkv[qcols]).reshape(128, 1),
                "bk": np.ascontiguousarray(bqkv[kcols]).reshape(128, 1),
                "bv": np.ascontiguousarray(bqkv[vcols]).reshape(128, 1),
                "bp": np.ascontiguousarray(
                    bproj[128 * c : 128 * (c + 1)]
                ).reshape(128, 1),
                "ident": identity,
                "ones": ones_col,
            }
        )
    return in_maps


def assemble(results):
    outT = np.concatenate([r["o"] for r in results], axis=0)  # [D, TOK]
    return np.ascontiguousarray(outT.T).reshape(B, N, D).astype(np.float32)


# Chosen dtype config (see build_nc docstring for the tradeoff).
CONFIG = {"qkv_dt": F32R, "attn_dt": F32R, "proj_dt": F32R}

_NC_CACHE = {}


def get_nc():
    if "nc" not in _NC_CACHE:
        _NC_CACHE["nc"] = build_nc(**CONFIG)
    return _NC_CACHE["nc"]


def kernel(x, Wqkv, bqkv, Wproj, bproj):
    nc = get_nc()
    in_maps = prep_in_maps(x, Wqkv, bqkv, Wproj, bproj, **CONFIG)
    res = run_bass_kernel_spmd(nc, in_maps, list(range(NCORES)))
    return assemble(res.results)



# revision 40
# speedup vs baseline: 1.3892x; 1.3892x over previous
"""Multi-head attention TRN2 kernel (B=2, N=2048, D=1024, H=16).

Sharding: tensor-parallel over heads. Each of the 8 cores owns 2 heads
(both batch elements) end-to-end through QKV projection and attention,
then per-head attention outputs are AllGathered in 512-token chunks and
each core computes a 128-column slice of the output projection.

Key performance structure (vs the v1 kernel):
- S^T matmuls contract over depth=64 only, so the two heads' S matmuls
  are issued back-to-back: they land on disjoint PE row-groups (rows
  0-63 vs 64-127 via base-partition-derived tile_position) and execute
  CONCURRENTLY, halving S cost.
- exp() runs on [128, 2, 512] score tiles (both heads of one key chunk)
  to amortize the ~352-cycle fixed ACTIVATE cost; the scalar engine is
  the attention-phase bottleneck (~1.15us per exp).
- Softmax normalization: the denominator row (from a ones-column in the
  PV matmul) is reciprocal'd with the fast approx DVE op, broadcast
  across 64 partitions with a tiny PE matmul (ones-vector outer
  product), and multiplied in - no 1-partition iterative reciprocal, no
  DRAM broadcast roundtrip.
- 8 fine-grained AllGathers (one per 512-token attention unit) so the
  gather pipeline starts early and the post-attention tail is short.
- QKV of batch 1 and the b=0 output projections are interleaved into
  the scalar-bound attention stream to keep the PE dense (HAM warm).
- x / Wqkv / Wproj / collective traffic in bf16 (halves DMA + enables
  fast weight load); attention operands f32r.

Self-contained: hardcodes shapes from the problem spec.
"""

import sys

for _p in ("/opt/trn_rl_repo", "/root/.axon_site/_ro/trn_rl_repo"):
    if _p not in sys.path:
        sys.path.append(_p)

import numpy as np
from contextlib import ExitStack

import concourse.bass as bass
import concourse.tile as tile
from concourse import mybir, bacc
from concourse.bass_utils import run_bass_kernel_spmd

F32 = mybir.dt.float32
F32R = mybir.dt.float32r
BF16 = mybir.dt.bfloat16
EXP = mybir.ActivationFunctionType.Exp

B = 2
N = 2048
D = 1024
H = 16
DEPTH = 64
TOK = B * N            # 4096 tokens total (both batches)
KC = D // 128          # 8 contraction chunks of 128
NBLK = TOK // 512      # 8 token blocks for streaming projections
SCALE = 1.0 / np.sqrt(DEPTH)
NCORES = 8
QBLK = 512             # query-block width in attention (= AllGather grain)
NU = N // QBLK         # 4 attention units per batch element
NJC = N // 128         # 16 key chunks per unit


def build_nc(reps=1, with_collective=True, qkv_dt=BF16, attn_dt=F32R,
             proj_dt=BF16, debug=False):
    """Build the per-core kernel program.

    reps>1 wraps the compute in a For_i hardware loop for benchmarking
    (collectives are skipped: they cannot appear inside control flow).

    qkv_dt: dtype of x^T and QKV weights (the QKV matmuls).
    attn_dt: dtype of q^T/k^T/V2/P^T (the S^T and P^T@V matmuls).
    proj_dt: dtype of the AllGathered A^T and Wproj (projection matmuls).
    """
    bench = reps > 1
    nc = bacc.Bacc(None)

    def dram_dt(dt):
        return F32 if dt == F32R else dt

    def cast(ap, dt):
        return ap.bitcast(F32R) if dt == F32R else ap

    xt = nc.dram_tensor("xt", [D, TOK], dram_dt(qkv_dt), kind="ExternalInput")
    wq = nc.dram_tensor("wq", [D, 128], dram_dt(qkv_dt), kind="ExternalInput")
    wk = nc.dram_tensor("wk", [D, 128], dram_dt(qkv_dt), kind="ExternalInput")
    wv = nc.dram_tensor("wv", [D, 128], dram_dt(qkv_dt), kind="ExternalInput")
    wp = nc.dram_tensor("wp", [D, 128], dram_dt(proj_dt), kind="ExternalInput")
    bq = nc.dram_tensor("bq", [128, 1], F32, kind="ExternalInput")
    bk = nc.dram_tensor("bk", [128, 1], F32, kind="ExternalInput")
    bv = nc.dram_tensor("bv", [128, 1], F32, kind="ExternalInput")
    bp = nc.dram_tensor("bp", [128, 1], F32, kind="ExternalInput")
    ident = nc.dram_tensor(
        "ident", [128, 128], dram_dt(attn_dt), kind="ExternalInput"
    )
    out = nc.dram_tensor("o", [128, TOK], F32, kind="ExternalOutput")

    # Collective staging: one AllGather per (b, 512-token unit).
    ag_in = [
        [nc.dram_tensor(f"agi{b}_{u}", [128, QBLK], proj_dt) for u in range(NU)]
        for b in range(B)
    ]
    ag_out = [
        [
            nc.dram_tensor(f"ago{b}_{u}", [D, QBLK], proj_dt,
                           addr_space="Shared")
            for u in range(NU)
        ]
        for b in range(B)
    ]

    dbg = {}
    if debug:
        for name, shape, dt in [
            ("dbg_qT", [128, 512], F32),
            ("dbg_kT", [128, 512], F32),
            ("dbg_v2ones", [128, 32], F32),
            ("dbg_v2data", [128, 130], F32),
            ("dbg_pt0", [128, 2, 512], F32),
            ("dbg_poA", [65, 512], F32),
            ("dbg_poB", [65, 512], F32),
            ("dbg_rc", [1, 2, 512], F32),
            ("dbg_bct", [64, 1024], F32),
            ("dbg_unr", [64, 2, 512], dram_dt(proj_dt)),
        ]:
            dbg[name] = nc.dram_tensor(name, shape, dt, kind="ExternalOutput")

    xt_r = xt.rearrange("(kc p) t -> p kc t", p=128)

    with tile.TileContext(nc) as tc, ExitStack() as ctx:
        wpool = ctx.enter_context(tc.tile_pool(name="w", bufs=1))
        qkpool = ctx.enter_context(tc.tile_pool(name="qk", bufs=1))
        vpool = ctx.enter_context(tc.tile_pool(name="v2", bufs=1))
        xpool = ctx.enter_context(tc.tile_pool(name="x", bufs=3))
        vtpool = ctx.enter_context(tc.tile_pool(name="vt", bufs=2))
        ptpool = ctx.enter_context(tc.tile_pool(name="pt", bufs=3))
        unpool = ctx.enter_context(tc.tile_pool(name="un", bufs=2))
        rcpool = ctx.enter_context(tc.tile_pool(name="rc", bufs=2))
        nrpool = ctx.enter_context(tc.tile_pool(name="nr", bufs=2))
        apool = ctx.enter_context(tc.tile_pool(name="ap", bufs=4))
        oupool = ctx.enter_context(tc.tile_pool(name="ou", bufs=2))
        # PSUM budget (8 banks of 2KB/partition):
        #   ps_ss (bufs=3): [128,2,512] f32 (2 banks each)        -> 6 banks
        #     (QKV q|k pairs, score pairs, proj chunk pairs, v-transpose
        #     staging, the denominator broadcast) - 3 bufs let the S^T
        #     stream run two key-chunks ahead of exp, so fill work never
        #     starves the scalar engine
        #   ps_po (bufs=2): [65,512] f32 po accumulators          -> 2 banks
        ps_ss = ctx.enter_context(tc.tile_pool(name="pss", bufs=3, space="PSUM"))
        ps_po = ctx.enter_context(tc.tile_pool(name="psp", bufs=2, space="PSUM"))

        # ---- weights / constants (loaded once, outside any rep loop) ----
        # Order matters: w_q + the first x block gate the first matmul.
        w_q = wpool.tile([128, KC, 128], qkv_dt, tag="w_q")
        w_k = wpool.tile([128, KC, 128], qkv_dt, tag="w_k")
        w_v = wpool.tile([128, KC, 128], qkv_dt, tag="w_v")
        w_p = wpool.tile([128, KC, 128], proj_dt, tag="w_p")
        nc.sync.dma_start(
            out=w_q, in_=cast(wq.rearrange("(kc p) m -> p kc m", p=128), qkv_dt)
        )
        nc.gpsimd.dma_start(
            out=w_k, in_=cast(wk.rearrange("(kc p) m -> p kc m", p=128), qkv_dt)
        )
        nc.scalar.dma_start(
            out=w_v, in_=cast(wv.rearrange("(kc p) m -> p kc m", p=128), qkv_dt)
        )
        b_q = wpool.tile([128, 1], F32, tag="b_q")
        b_k = wpool.tile([128, 1], F32, tag="b_k")
        b_v = wpool.tile([128, 1], F32, tag="b_v")
        b_p = wpool.tile([128, 1], F32, tag="b_p")
        for t, src in ((b_q, bq), (b_k, bk), (b_v, bv)):
            nc.scalar.dma_start(out=t, in_=src[:])
        id_t = wpool.tile([128, 128], attn_dt, tag="id_t")
        nc.scalar.dma_start(out=id_t, in_=cast(ident[:], attn_dt))

        # qT/kT: [feature 128 (= 2 heads x 64), token 4096]; head hl in rows
        # hl*64:(hl+1)*64 so both S^T operands share a partition base.
        qT = qkpool.tile([128, TOK], attn_dt, tag="qT")
        kT = qkpool.tile([128, TOK], attn_dt, tag="kT")
        # V2: [token part, 32 token-chunks, 130]: v_h0 | ones | v_h1 | ones
        V2 = vpool.tile([128, TOK // 128, 130], attn_dt, tag="V2")
        # memset can't emit f32r; stage fp32 ones and round via tensor_copy
        onesF = wpool.tile([128, 64], F32, tag="onesF")
        nc.vector.memset(onesF, 1.0)
        nc.vector.tensor_copy(out=V2[:, :, 64], in_=onesF[:, 0:32])
        nc.vector.tensor_copy(out=V2[:, :, 129], in_=onesF[:, 0:32])
        # ones row at partition 64 for the denominator broadcast matmuls
        onesr = wpool.tile([128, 64], attn_dt, tag="onesr")
        nc.vector.tensor_copy(out=onesr[64:65, :], in_=onesF[64:65, :])

        # w_p and b_p are only needed in phase D; load them late so they
        # don't delay the QKV-critical transfers.
        nc.scalar.dma_start(out=b_p, in_=bp[:])
        nc.scalar.dma_start(
            out=w_p, in_=cast(wp.rearrange("(kc p) m -> p kc m", p=128), proj_dt)
        )

        rep_ctx = ExitStack()
        if bench:
            rep_ctx.enter_context(
                tc.For_i(
                    0,
                    reps,
                    1,
                    hint_engines=(
                        mybir.EngineType.PE,
                        mybir.EngineType.Activation,
                        mybir.EngineType.DVE,
                        mybir.EngineType.SP,
                    ),
                )
            )

        # ================= QKV projection for one 512-token block ==========
        def qkv_dma(blk):
            t0 = blk * 512
            xb = xpool.tile([128, KC, 512], qkv_dt, tag="xb")
            if blk < 4:
                # b=0 QKV is DMA-paced: split every block across two queues
                nc.sync.dma_start(
                    out=xb[:, 0:4, :], in_=cast(xt_r[:, 0:4, t0 : t0 + 512], qkv_dt)
                )
                nc.gpsimd.dma_start(
                    out=xb[:, 4:8, :], in_=cast(xt_r[:, 4:8, t0 : t0 + 512], qkv_dt)
                )
            else:
                eng = nc.sync if blk % 2 == 0 else nc.gpsimd
                eng.dma_start(out=xb, in_=cast(xt_r[:, :, t0 : t0 + 512], qkv_dt))
            return xb

        def qkv_fills(blk, xbs, st=None):
            """Return fine-grained closures (~1-2us of PE work each) that
            together compute QKV for one block; for splicing into the
            scalar-bound attention stream without stalling the exp feed."""
            t0 = blk * 512
            st = {} if st is None else st

            def mm_group(pskey, sl, w_t, kcs):
                def go():
                    if pskey not in st:
                        st[pskey] = ps_ss.tile([128, 2, 512], F32, tag="ss", name=f"qkv_{pskey}")
                    for kc in kcs:
                        nc.tensor.matmul(
                            out=st[pskey][:, sl, :],
                            lhsT=w_t[:, kc, :],
                            rhs=xbs[blk][:, kc, :],
                            start=(kc == 0),
                            stop=(kc == KC - 1),
                        )
                return go

            def qk_adds():
                nc.vector.tensor_scalar_add(
                    out=qT[:, t0 : t0 + 512], in0=st["qk"][:, 0, :], scalar1=b_q
                )
                nc.vector.tensor_scalar_add(
                    out=kT[:, t0 : t0 + 512], in0=st["qk"][:, 1, :], scalar1=b_k
                )

            def v_trans():
                vtmp = vtpool.tile([128, 512], attn_dt, tag="vtmp")
                nc.vector.tensor_scalar_add(
                    out=vtmp, in0=st["v"][:, 0, :], scalar1=b_v
                )
                bct = ps_ss.tile([128, 2, 512], F32, tag="ss", name="vt_ps")
                st["bct"] = bct
                for s in range(4):
                    nc.tensor.transpose(
                        out=bct[:, 0, s * 128 : (s + 1) * 128].bitcast(attn_dt),
                        in_=vtmp[:, s * 128 : (s + 1) * 128],
                        identity=id_t,
                    )

            def v_copies():
                for s in range(4):
                    ch = blk * 4 + s
                    src = st["bct"][:, 0, s * 128 : (s + 1) * 128].bitcast(attn_dt)
                    nc.vector.tensor_copy(out=V2[:, ch, 0:64], in_=src[:, 0:64])
                    nc.vector.tensor_copy(
                        out=V2[:, ch, 65:129], in_=src[:, 64:128]
                    )

            return [
                mm_group("qk", 0, w_q, range(0, 4)),
                mm_group("qk", 0, w_q, range(4, 8)),
                mm_group("qk", 1, w_k, range(0, 4)),
                mm_group("qk", 1, w_k, range(4, 8)),
                qk_adds,
                mm_group("v", 0, w_v, range(0, 4)),
                mm_group("v", 0, w_v, range(4, 8)),
                v_trans,
                v_copies,
            ]

        def qkv_block(blk):
            xb = qkv_dma(blk)
            for f in qkv_fills(blk, {blk: xb}):
                f()

        # ============== attention unit: (b, 512 queries, both heads) =======
        def attention_unit(b, u, fill):
            """fill: list of closures to emit between jc steps (PE filler
            work for the scalar-bound stretch: late QKV blocks, early
            projections)."""
            i0 = b * N + u * QBLK
            is_dbg = debug and b == 0 and u == 0
            if is_dbg:
                nc.sync.dma_start(out=dbg["dbg_qT"][:], in_=qT[:, 0:512].bitcast(F32))
                nc.sync.dma_start(out=dbg["dbg_kT"][:], in_=kT[:, 0:512].bitcast(F32))
                nc.sync.dma_start(out=dbg["dbg_v2ones"][:], in_=V2[:, :, 64].bitcast(F32))
                nc.sync.dma_start(out=dbg["dbg_v2data"][:], in_=V2[:, 0, :].bitcast(F32))
            poA = ps_po.tile([65, QBLK], F32, tag="po")
            poB = ps_po.tile([65, QBLK], F32, tag="po")

            def s_step(jc):
                j0 = b * N + jc * 128
                ss = ps_ss.tile([128, 2, QBLK], F32, tag="ss")
                for hl in range(2):
                    hs = hl * 64
                    nc.tensor.matmul(
                        out=ss[:, hl, :],
                        lhsT=kT[hs : hs + 64, j0 : j0 + 128],
                        rhs=qT[hs : hs + 64, i0 : i0 + QBLK],
                        start=True,
                        stop=True,
                    )
                pt = ptpool.tile([128, 2, QBLK], attn_dt, tag="pt")
                nc.scalar.activation(out=pt, in_=ss, func=EXP, scale=float(SCALE))
                return pt

            # software pipeline: S^T one step ahead of PV so the PE never
            # waits on ScalarE's exp
            pts = s_step(0)
            if is_dbg:
                nc.sync.dma_start(out=dbg["dbg_pt0"][:], in_=pts.bitcast(F32))
            fi = 0
            for jc in range(NJC):
                pt_cur = pts
                if jc + 1 < NJC:
                    pts = s_step(jc + 1)
                if fi < len(fill):
                    fill[fi]()
                    fi += 1
                for hl, po in ((0, poA), (1, poB)):
                    nc.tensor.matmul(
                        out=po,
                        lhsT=V2[:, b * (N // 128) + jc, hl * 65 : hl * 65 + 65],
                        rhs=pt_cur[:, hl, :],
                        start=(jc == 0),
                        stop=(jc == NJC - 1),
                    )
            for f in fill[fi:]:
                f()
            if is_dbg:
                dstA = unpool.tile([65, 512], F32, tag="dbgA")
                dstB = unpool.tile([65, 512], F32, tag="dbgB")
                nc.vector.tensor_copy(out=dstA, in_=poA)
                nc.vector.tensor_copy(out=dstB, in_=poB)
                nc.sync.dma_start(out=dbg["dbg_poA"][:], in_=dstA)
                nc.sync.dma_start(out=dbg["dbg_poB"][:], in_=dstB)

            # ---- evict: normalize and stage for the AllGather ----
            unS = unpool.tile([64, 2, QBLK], F32, tag="un")
            rc = rcpool.tile([128, 2, QBLK], F32, tag="rc")
            rc2 = rcpool.tile([128, 2, QBLK], attn_dt, tag="rc2")
            bct = ps_ss.tile([128, 2, QBLK], F32, tag="ss", name="bc_ps")
            dn = rcpool.tile([128, 2, QBLK], F32, tag="dn")
            for hl, po in ((0, poA), (1, poB)):
                nc.vector.tensor_copy(out=unS[:, hl, :], in_=po[0:64, :])
                nc.vector.tensor_copy(out=dn[64:65, hl, :], in_=po[64:65, :])
            # custom DVE ops require base_partition 0: run the reciprocal
            # over the full partition range (same per-lane cost; only row
            # 64 is meaningful downstream)
            nc.vector.reciprocal_approx_fast(out=rc, in_=dn)
            # round the fp32 reciprocals to the matmul dtype (verifier
            # requires f32r matmul operands to be f32r-rounded at source)
            nc.vector.tensor_copy(out=rc2[64:65, :, :], in_=rc[64:65, :, :])
            if is_dbg:
                nc.sync.dma_start(out=dbg["dbg_rc"][:], in_=rc[64:65, :, :])
            for hl in range(2):
                # [64, 512] = 1/denom broadcast via ones-vector outer product
                nc.tensor.matmul(
                    out=bct[0:64, hl, :],
                    lhsT=onesr[64:65, :],
                    rhs=rc2[64:65, hl, :],
                    start=True,
                    stop=True,
                )
            if is_dbg:
                dstC = unpool.tile([64, 2, 512], F32, tag="dbgC")
                nc.vector.tensor_copy(out=dstC, in_=bct[0:64, :, :])
                nc.sync.dma_start(out=dbg["dbg_bct"][:], in_=dstC)
            unr = nrpool.tile([64, 2, QBLK], proj_dt, tag="unr")
            for hl in range(2):
                nc.vector.tensor_mul(
                    out=unr[:, hl, :],
                    in0=unS[:, hl, :],
                    in1=bct[0:64, hl, :],
                )
                nc.gpsimd.dma_start(
                    out=ag_in[b][u][hl * 64 : hl * 64 + 64, :],
                    in_=unr[:, hl, :],
                )
            if is_dbg:
                nc.sync.dma_start(out=dbg["dbg_unr"][:], in_=unr)
            if with_collective:
                nc.gpsimd.collective_compute(
                    "AllGather",
                    mybir.AluOpType.bypass,
                    ins=[ag_in[b][u][:]],
                    outs=[ag_out[b][u][:]],
                    replica_groups=[list(range(NCORES))],
                )

        # ========= output projection for a pair of 512-token units =========
        def proj_dma(b, u, split=False):
            ag_r = ag_out[b][u].rearrange("(kc p) t -> p kc t", p=128)
            ab = apool.tile([128, KC, 512], proj_dt, tag="ab")
            if split:
                # tail load: split across two idle queues (a single DMA
                # queue moves only ~45GB/s). Never the scalar queue - a
                # pending-dep DMA trigger head-of-line blocks the exps.
                nc.sync.dma_start(out=ab[:, 0:4, :], in_=ag_r[:, 0:4, :])
                nc.gpsimd.dma_start(out=ab[:, 4:8, :], in_=ag_r[:, 4:8, :])
            else:
                nc.sync.dma_start(out=ab, in_=ag_r)
            return ab

        def proj_fills(b, up, abt):
            st = {}

            def mm_group(j, kcs):
                def go():
                    if "ps" not in st:
                        st["ps"] = ps_ss.tile([128, 2, 512], F32, tag="ss", name="proj_ps")
                    for kc in kcs:
                        nc.tensor.matmul(
                            out=st["ps"][:, j, :],
                            lhsT=w_p[:, kc, :],
                            rhs=abt[(b, up * 2 + j)][:, kc, :],
                            start=(kc == 0),
                            stop=(kc == KC - 1),
                        )
                return go

            def evict():
                ot = oupool.tile([128, 2, 512], F32, tag="ot")
                nc.vector.tensor_scalar_add(out=ot, in0=st["ps"], scalar1=b_p)
                t0 = b * N + up * 1024
                nc.sync.dma_start(out=out[:, t0 : t0 + 1024], in_=ot)

            return [
                mm_group(0, range(0, 4)),
                mm_group(0, range(4, 8)),
                mm_group(1, range(0, 4)),
                mm_group(1, range(4, 8)),
                evict,
            ]

        def proj_compute(b, up, abt):
            for f in proj_fills(b, up, abt):
                f()

        # =========================== schedule ==============================
        # QKV for b=0; b=0 attention with b=1 QKV blocks spliced into the
        # scalar-bound stretches as fine-grained (~1-2us) closures so the
        # S->exp feed is never starved; b=1 attention with the b=0 output
        # projections spliced in the same way; then the tail projections.
        for blk in range(4):
            qkv_block(blk)
        xbs = {}
        abt = {}
        xbs[4] = qkv_dma(4)
        nop = lambda: None
        for u in range(NU):
            blk = 4 + u
            fill = []
            if blk + 1 < NBLK:
                fill.append(lambda b2=blk + 1: xbs.__setitem__(b2, qkv_dma(b2)))
            if u == 0:
                # xb4's DMA was issued just above; give it a few jc steps
                # of headroom before the PE consumes it
                fill.extend([nop, nop, nop])
            fill.extend(qkv_fills(blk, xbs))
            attention_unit(0, u, fill)
        for u in range(NU):
            fill = []
            if u == 0:
                fill = [
                    lambda: abt.__setitem__((0, 0), proj_dma(0, 0)),
                    lambda: abt.__setitem__((0, 1), proj_dma(0, 1)),
                ]
            elif u == 1:
                fill = [
                    lambda: abt.__setitem__((0, 2), proj_dma(0, 2)),
                    lambda: abt.__setitem__((0, 3), proj_dma(0, 3)),
                ] + proj_fills(0, 0, abt)
            elif u == 2:
                fill = [
                    lambda: abt.__setitem__((1, 0), proj_dma(1, 0)),
                    lambda: abt.__setitem__((1, 1), proj_dma(1, 1)),
                ] + proj_fills(0, 1, abt)
            else:
                fill = [
                    lambda: abt.__setitem__((1, 2), proj_dma(1, 2)),
                ] + proj_fills(1, 0, abt)
            attention_unit(1, u, fill)
        abt[(1, 3)] = proj_dma(1, 3, split=True)
        proj_compute(1, 1, abt)

        rep_ctx.close()

    nc.compile()
    return nc


def np_dt(dt):
    return mybir.dt.np(F32 if dt == F32R else dt)


def prep_in_maps(x, Wqkv, bqkv, Wproj, bproj, qkv_dt=BF16, attn_dt=F32R,
                 proj_dt=BF16):
    x = np.asarray(x, dtype=np.float32)
    Wqkv = np.asarray(Wqkv, dtype=np.float32)
    bqkv = np.asarray(bqkv, dtype=np.float32)
    Wproj = np.asarray(Wproj, dtype=np.float32)
    bproj = np.asarray(bproj, dtype=np.float32)

    xtn = np.ascontiguousarray(x.reshape(TOK, D).T).astype(np_dt(qkv_dt))
    identity = np.eye(128, dtype=np_dt(attn_dt))

    # AllGather output rows are rank-major: row c*128 + hl*64 + d holds
    # feature (2c+hl)*64 + d; permute Wproj's contraction rows to match.
    wp_row_perm = np.empty(D, dtype=np.int64)
    for cc in range(NCORES):
        for hlhl in range(2):
            rows = np.arange(64)
            wp_row_perm[cc * 128 + hlhl * 64 + rows] = (2 * cc + hlhl) * 64 + rows

    # qkv column index for (head h, depth d, which): h*192 + d*3 + which
    d_idx = np.arange(DEPTH)
    in_maps = []
    for c in range(NCORES):
        h0, h1 = 2 * c, 2 * c + 1
        qcols = np.concatenate([h0 * 192 + 3 * d_idx, h1 * 192 + 3 * d_idx])
        kcols = qcols + 1
        vcols = qcols + 2
        in_maps.append(
            {
                "xt": xtn,
                "wq": np.ascontiguousarray(Wqkv[:, qcols]).astype(np_dt(qkv_dt)),
                "wk": np.ascontiguousarray(Wqkv[:, kcols]).astype(np_dt(qkv_dt)),
                "wv": np.ascontiguousarray(Wqkv[:, vcols]).astype(np_dt(qkv_dt)),
                "wp": np.ascontiguousarray(
                    Wproj[wp_row_perm, 128 * c : 128 * (c + 1)]
                ).astype(np_dt(proj_dt)),
                "bq": np.ascontiguousarray(bqkv[qcols]).reshape(128, 1),
                "bk": np.ascontiguousarray(bqkv[kcols]).reshape(128, 1),
                "bv": np.ascontiguousarray(bqkv[vcols]).reshape(128, 1),
                "bp": np.ascontiguousarray(
                    bproj[128 * c : 128 * (c + 1)]
                ).reshape(128, 1),
                "ident": identity,
            }
        )
    return in_maps


def assemble(results):
    outT = np.concatenate([r["o"] for r in results], axis=0)  # [D, TOK]
    return np.ascontiguousarray(outT.T).reshape(B, N, D).astype(np.float32)


# Chosen dtype config (see build_nc docstring for the tradeoff).
CONFIG = {"qkv_dt": BF16, "attn_dt": F32R, "proj_dt": BF16}

_NC_CACHE = {}


def get_nc():
    if "nc" not in _NC_CACHE:
        _NC_CACHE["nc"] = build_nc(**CONFIG)
    return _NC_CACHE["nc"]


def kernel(x, Wqkv, bqkv, Wproj, bproj):
    nc = get_nc()
    in_maps = prep_in_maps(x, Wqkv, bqkv, Wproj, bproj, **CONFIG)
    res = run_bass_kernel_spmd(nc, in_maps, list(range(NCORES)))
    return assemble(res.results)


# revision 71
# speedup vs baseline: 1.4665x; 1.0556x over previous
"""Multi-head attention TRN2 kernel (B=2, N=2048, D=1024, H=16).

Sharding: tensor-parallel over heads. Each of the 8 cores owns 2 heads
(both batch elements) end-to-end through QKV projection and attention,
then per-head attention outputs are AllGathered in 512-token chunks and
each core computes a 128-column slice of the output projection.

Key performance structure (vs the v1 kernel):
- S^T matmuls contract over depth=64 only, so the two heads' S matmuls
  are issued back-to-back: they land on disjoint PE row-groups (rows
  0-63 vs 64-127 via base-partition-derived tile_position) and execute
  CONCURRENTLY, halving S cost. The S^T stream runs two key chunks
  ahead of exp so fill work never starves the scalar engine.
- exp() runs on [128, 2, 512] score tiles (both heads of one key chunk)
  to amortize the ~352-cycle fixed ACTIVATE cost; the scalar engine
  (~1.15us per exp, ~147us total) and the PE are closely balanced.
- Softmax normalization: the denominator row (from a ones-column in the
  PV matmul) is reciprocal'd with the fast approx DVE op (base
  partition 0 only!), broadcast across 64 partitions with a tiny PE
  matmul (ones outer product), and multiplied in - no 1-partition
  iterative reciprocal, no DRAM broadcast roundtrip. The PE part of
  each evict is deferred into the next unit's stream.
- Collectives: the CC stream serializes, so b=0 uses two 1024-token
  AllGathers (low stream occupancy) and b=1 four 512-token ones (small
  tail). b=1 units run in order 0,2,3,1 so only the final unit's
  gather is outstanding at the end.
- Almost all QKV work and the output projections ride as fine-grained
  (~1-2us) fill closures inside the attention units' jc loops, always
  emitted before their consumers (emission order IS dependency order).
- DMA: a single queue moves only ~45GB/s, so startup x-blocks split
  3 ways (sync/gpsimd/scalar) and readbacks 2 ways. AllGather-dependent
  DMA triggers never go on the scalar queue: the Tile scheduler may
  hoist them into the exp stream where their wait blocks the engine.
- dtypes: x/Wqkv/Wproj/collective traffic bf16 (halves DMA + fast
  weight load); attention operands fp16 (1cyc/row like bf16 but 10
  mantissa bits, and FWL-capable unlike f32r); all psum accumulation
  fp32. Scaled absmax error ~4e-3.

Self-contained: hardcodes shapes from the problem spec.
"""

import sys

for _p in ("/opt/trn_rl_repo", "/root/.axon_site/_ro/trn_rl_repo"):
    if _p not in sys.path:
        sys.path.append(_p)

import numpy as np
from contextlib import ExitStack

import concourse.bass as bass
import concourse.tile as tile
from concourse import mybir, bacc
from concourse.bass_utils import run_bass_kernel_spmd

F32 = mybir.dt.float32
F32R = mybir.dt.float32r
BF16 = mybir.dt.bfloat16
F16 = mybir.dt.float16
EXP = mybir.ActivationFunctionType.Exp

B = 2
N = 2048
D = 1024
H = 16
DEPTH = 64
TOK = B * N            # 4096 tokens total (both batches)
KC = D // 128          # 8 contraction chunks of 128
NBLK = TOK // 512      # 8 token blocks for streaming projections
SCALE = 1.0 / np.sqrt(DEPTH)
NCORES = 8
QBLK = 512             # query-block width in attention (= AllGather grain)
NU = N // QBLK         # 4 attention units per batch element
NJC = N // 128         # 16 key chunks per unit


def build_nc(reps=1, with_collective=True, qkv_dt=BF16, attn_dt=F32R,
             proj_dt=BF16, debug=False):
    """Build the per-core kernel program.

    reps>1 wraps the compute in a For_i hardware loop for benchmarking
    (collectives are skipped: they cannot appear inside control flow).

    qkv_dt: dtype of x^T and QKV weights (the QKV matmuls).
    attn_dt: dtype of q^T/k^T/V2/P^T (the S^T and P^T@V matmuls).
    proj_dt: dtype of the AllGathered A^T and Wproj (projection matmuls).
    """
    bench = reps > 1
    nc = bacc.Bacc(None)

    def dram_dt(dt):
        return F32 if dt == F32R else dt

    def cast(ap, dt):
        return ap.bitcast(F32R) if dt == F32R else ap

    xt = nc.dram_tensor("xt", [D, TOK], dram_dt(qkv_dt), kind="ExternalInput")
    wq = nc.dram_tensor("wq", [D, 128], dram_dt(qkv_dt), kind="ExternalInput")
    wk = nc.dram_tensor("wk", [D, 128], dram_dt(qkv_dt), kind="ExternalInput")
    wv = nc.dram_tensor("wv", [D, 128], dram_dt(qkv_dt), kind="ExternalInput")
    wp = nc.dram_tensor("wp", [D, 128], dram_dt(proj_dt), kind="ExternalInput")
    bq = nc.dram_tensor("bq", [128, 1], F32, kind="ExternalInput")
    bk = nc.dram_tensor("bk", [128, 1], F32, kind="ExternalInput")
    bv = nc.dram_tensor("bv", [128, 1], F32, kind="ExternalInput")
    bp = nc.dram_tensor("bp", [128, 1], F32, kind="ExternalInput")
    ident = nc.dram_tensor(
        "ident", [128, 128], dram_dt(attn_dt), kind="ExternalInput"
    )
    out = nc.dram_tensor("o", [128, TOK], F32, kind="ExternalOutput")

    # Collective staging. The CC stream serializes all collectives, so b=0
    # (whose gathers have slack) uses two 1024-token AllGathers while b=1
    # (whose last gather is tail-critical) uses four 512-token ones - this
    # keeps total stream occupancy low AND the final gather small.
    ag_in0 = [nc.dram_tensor(f"agi0_{h}", [128, 2 * QBLK], proj_dt)
              for h in range(2)]
    ag_out0 = [
        nc.dram_tensor(f"ago0_{h}", [D, 2 * QBLK], proj_dt, addr_space="Shared")
        for h in range(2)
    ]
    ag_in1 = [nc.dram_tensor(f"agi1_{u}", [128, QBLK], proj_dt)
              for u in range(NU)]
    ag_out1 = [
        nc.dram_tensor(f"ago1_{u}", [D, QBLK], proj_dt, addr_space="Shared")
        for u in range(NU)
    ]

    dbg = {}
    if debug:
        for name, shape, dt in [
            ("dbg_qT", [128, 512], dram_dt(attn_dt)),
            ("dbg_kT", [128, 512], dram_dt(attn_dt)),
            ("dbg_v2ones", [128, 32], dram_dt(attn_dt)),
            ("dbg_v2data", [128, 130], dram_dt(attn_dt)),
            ("dbg_pt0", [128, 2, 512], dram_dt(attn_dt)),
            ("dbg_poA", [65, 512], F32),
            ("dbg_poB", [65, 512], F32),
            ("dbg_rc", [1, 2, 512], F32),
            ("dbg_bct", [64, 1024], F32),
            ("dbg_unr", [64, 2, 512], dram_dt(proj_dt)),
        ]:
            dbg[name] = nc.dram_tensor(name, shape, dt, kind="ExternalOutput")

    xt_r = xt.rearrange("(kc p) t -> p kc t", p=128)

    with tile.TileContext(nc) as tc, ExitStack() as ctx:
        wpool = ctx.enter_context(tc.tile_pool(name="w", bufs=1))
        qkpool = ctx.enter_context(tc.tile_pool(name="qk", bufs=1))
        vpool = ctx.enter_context(tc.tile_pool(name="v2", bufs=1))
        xpool = ctx.enter_context(tc.tile_pool(name="x", bufs=5))
        vtpool = ctx.enter_context(tc.tile_pool(name="vt", bufs=2))
        ptpool = ctx.enter_context(tc.tile_pool(name="pt", bufs=4))
        unpool = ctx.enter_context(tc.tile_pool(name="un", bufs=2))
        rcpool = ctx.enter_context(tc.tile_pool(name="rc", bufs=2))
        nrpool = ctx.enter_context(tc.tile_pool(name="nr", bufs=2))
        apool = ctx.enter_context(tc.tile_pool(name="ap", bufs=4))
        oupool = ctx.enter_context(tc.tile_pool(name="ou", bufs=2))
        # PSUM budget (8 banks of 2KB/partition):
        #   ps_ss (bufs=3): [128,2,512] f32 (2 banks each)        -> 6 banks
        #     (QKV q|k pairs, score pairs, proj chunk pairs, v-transpose
        #     staging, the denominator broadcast) - 3 bufs let the S^T
        #     stream run two key-chunks ahead of exp, so fill work never
        #     starves the scalar engine
        #   ps_po (bufs=2): [65,512] f32 po accumulators          -> 2 banks
        ps_ss = ctx.enter_context(tc.tile_pool(name="pss", bufs=3, space="PSUM"))
        ps_po = ctx.enter_context(tc.tile_pool(name="psp", bufs=2, space="PSUM"))

        # ---- weights / constants (loaded once, outside any rep loop) ----
        # Order matters: w_q + the first x block gate the first matmul.
        # w_q is split across two queues so its first half (all the first
        # q matmul group needs) lands in ~3us; w_k/w_v follow the x thirds
        # on their queues since they gate only the later k/v groups.
        w_q = wpool.tile([128, KC, 128], qkv_dt, tag="w_q")
        w_k = wpool.tile([128, KC, 128], qkv_dt, tag="w_k")
        w_v = wpool.tile([128, KC, 128], qkv_dt, tag="w_v")
        w_p = wpool.tile([128, KC, 128], proj_dt, tag="w_p")
        wq_r = wq.rearrange("(kc p) m -> p kc m", p=128)
        nc.sync.dma_start(out=w_q[:, 0:4, :], in_=cast(wq_r[:, 0:4, :], qkv_dt))
        nc.gpsimd.dma_start(out=w_q[:, 4:8, :], in_=cast(wq_r[:, 4:8, :], qkv_dt))
        b_q = wpool.tile([128, 1], F32, tag="b_q")
        b_k = wpool.tile([128, 1], F32, tag="b_k")
        b_v = wpool.tile([128, 1], F32, tag="b_v")
        b_p = wpool.tile([128, 1], F32, tag="b_p")
        for t, src in ((b_q, bq), (b_k, bk), (b_v, bv)):
            nc.scalar.dma_start(out=t, in_=src[:])
        id_t = wpool.tile([128, 128], attn_dt, tag="id_t")
        nc.scalar.dma_start(out=id_t, in_=cast(ident[:], attn_dt))

        # qT/kT: [feature 128 (= 2 heads x 64), token 4096]; head hl in rows
        # hl*64:(hl+1)*64 so both S^T operands share a partition base.
        qT = qkpool.tile([128, TOK], attn_dt, tag="qT")
        kT = qkpool.tile([128, TOK], attn_dt, tag="kT")
        # V2: [token part, 32 token-chunks, 130]: v_h0 | ones | v_h1 | ones
        V2 = vpool.tile([128, TOK // 128, 130], attn_dt, tag="V2")
        # memset can't emit f32r; stage fp32 ones and round via tensor_copy
        onesF = wpool.tile([128, 64], F32, tag="onesF")
        nc.vector.memset(onesF, 1.0)
        nc.vector.tensor_copy(out=V2[:, :, 64], in_=onesF[:, 0:32])
        nc.vector.tensor_copy(out=V2[:, :, 129], in_=onesF[:, 0:32])
        # ones row at partition 64 for the denominator broadcast matmuls
        onesr = wpool.tile([128, 64], attn_dt, tag="onesr")
        nc.vector.tensor_copy(out=onesr[64:65, :], in_=onesF[64:65, :])


        rep_ctx = ExitStack()
        if bench:
            rep_ctx.enter_context(
                tc.For_i(
                    0,
                    reps,
                    1,
                    hint_engines=(
                        mybir.EngineType.PE,
                        mybir.EngineType.Activation,
                        mybir.EngineType.DVE,
                        mybir.EngineType.SP,
                    ),
                )
            )

        # ================= QKV projection for one 512-token block ==========
        def qkv_dma(blk):
            t0 = blk * 512
            xb = xpool.tile([128, KC, 512], qkv_dt, tag="xb")
            # a single DMA queue moves only ~45GB/s: split across queues.
            # The upfront b=0 blocks are DMA-critical: 3-way split (the
            # scalar queue is free before the exps start). Later blocks
            # are prefetched with a full unit of lead: 2-way is enough.
            if blk < 4:
                nc.sync.dma_start(
                    out=xb[:, 0:3, :], in_=cast(xt_r[:, 0:3, t0 : t0 + 512], qkv_dt)
                )
                nc.gpsimd.dma_start(
                    out=xb[:, 3:6, :], in_=cast(xt_r[:, 3:6, t0 : t0 + 512], qkv_dt)
                )
                nc.scalar.dma_start(
                    out=xb[:, 6:8, :], in_=cast(xt_r[:, 6:8, t0 : t0 + 512], qkv_dt)
                )
            else:
                nc.sync.dma_start(
                    out=xb[:, 0:4, :], in_=cast(xt_r[:, 0:4, t0 : t0 + 512], qkv_dt)
                )
                nc.gpsimd.dma_start(
                    out=xb[:, 4:8, :], in_=cast(xt_r[:, 4:8, t0 : t0 + 512], qkv_dt)
                )
            return xb

        def qkv_fills(blk, xbs, st=None):
            """Return fine-grained closures (~1-2us of PE work each) that
            together compute QKV for one block; for splicing into the
            scalar-bound attention stream without stalling the exp feed."""
            t0 = blk * 512
            st = {} if st is None else st

            def mm_group(pskey, sl, w_t, kcs):
                def go():
                    if pskey not in st:
                        st[pskey] = ps_ss.tile([128, 2, 512], F32, tag="ss", name=f"qkv_{pskey}")
                    for kc in kcs:
                        nc.tensor.matmul(
                            out=st[pskey][:, sl, :],
                            lhsT=w_t[:, kc, :],
                            rhs=xbs[blk][:, kc, :],
                            start=(kc == 0),
                            stop=(kc == KC - 1),
                        )
                return go

            def qk_adds():
                nc.vector.tensor_scalar_add(
                    out=qT[:, t0 : t0 + 512], in0=st["qk"][:, 0, :], scalar1=b_q
                )
                nc.vector.tensor_scalar_add(
                    out=kT[:, t0 : t0 + 512], in0=st["qk"][:, 1, :], scalar1=b_k
                )

            def v_trans():
                vtmp = vtpool.tile([128, 512], attn_dt, tag="vtmp")
                nc.vector.tensor_scalar_add(
                    out=vtmp, in0=st["v"][:, 0, :], scalar1=b_v
                )
                # transpose back into slot 1 of the same psum tile (vtmp
                # holds the slot-0 data already): avoids a third ss-pool
                # allocation per block, letting consecutive QKV blocks
                # pipeline within 3 bufs
                wd = 128 if attn_dt == F32R else 64  # f32-cols per transpose
                for s in range(4):
                    nc.tensor.transpose(
                        out=st["v"][:, 1, s * wd : (s + 1) * wd].bitcast(attn_dt),
                        in_=vtmp[:, s * 128 : (s + 1) * 128],
                        identity=id_t,
                    )

            def v_copies():
                wd = 128 if attn_dt == F32R else 64
                for s in range(4):
                    ch = blk * 4 + s
                    src = st["v"][:, 1, s * wd : (s + 1) * wd].bitcast(attn_dt)
                    nc.vector.tensor_copy(out=V2[:, ch, 0:64], in_=src[:, 0:64])
                    nc.vector.tensor_copy(
                        out=V2[:, ch, 65:129], in_=src[:, 64:128]
                    )

            return [
                mm_group("qk", 0, w_q, range(0, 4)),
                mm_group("qk", 0, w_q, range(4, 8)),
                mm_group("qk", 1, w_k, range(0, 4)),
                mm_group("qk", 1, w_k, range(4, 8)),
                qk_adds,
                mm_group("v", 0, w_v, range(0, 4)),
                mm_group("v", 0, w_v, range(4, 8)),
                v_trans,
                v_copies,
            ]

        def qkv_block(blk):
            xb = qkv_dma(blk)
            for f in qkv_fills(blk, {blk: xb}):
                f()

        # -- finer-grained variants: k+v (needed as attention keys one unit
        # ahead) separately from q (needed as queries ~two units later) --
        def kv_fills(blk, xbs):
            """k into slot 0 and v into slot 1 of one psum tile; the v
            transposes reuse slot 0 after the k bias-add has drained it."""
            t0 = blk * 512
            st = {}

            def mm(sl, w_t, kcs):
                def go():
                    if "kv" not in st:
                        st["kv"] = ps_ss.tile([128, 2, 512], F32, tag="ss",
                                              name="kv_ps")
                    for kc in kcs:
                        nc.tensor.matmul(
                            out=st["kv"][:, sl, :],
                            lhsT=w_t[:, kc, :],
                            rhs=xbs[blk][:, kc, :],
                            start=(kc == 0),
                            stop=(kc == KC - 1),
                        )
                return go

            def k_add():
                nc.vector.tensor_scalar_add(
                    out=kT[:, t0 : t0 + 512], in0=st["kv"][:, 0, :], scalar1=b_k
                )

            def v_trans():
                vtmp = vtpool.tile([128, 512], attn_dt, tag="vtmp")
                nc.vector.tensor_scalar_add(
                    out=vtmp, in0=st["kv"][:, 1, :], scalar1=b_v
                )
                wd = 128 if attn_dt == F32R else 64
                for s in range(4):
                    nc.tensor.transpose(
                        out=st["kv"][:, 0, s * wd : (s + 1) * wd].bitcast(attn_dt),
                        in_=vtmp[:, s * 128 : (s + 1) * 128],
                        identity=id_t,
                    )

            def v_copies():
                wd = 128 if attn_dt == F32R else 64
                for s in range(4):
                    ch = blk * 4 + s
                    src = st["kv"][:, 0, s * wd : (s + 1) * wd].bitcast(attn_dt)
                    nc.vector.tensor_copy(out=V2[:, ch, 0:64], in_=src[:, 0:64])
                    nc.vector.tensor_copy(
                        out=V2[:, ch, 65:129], in_=src[:, 64:128]
                    )

            return [
                mm(0, w_k, range(0, 4)),
                mm(0, w_k, range(4, 8)),
                k_add,
                mm(1, w_v, range(0, 4)),
                mm(1, w_v, range(4, 8)),
                v_trans,
                v_copies,
            ]

        def q2_fills(blkA, blkB, xbs):
            """q projections for two blocks sharing one psum tile."""
            st = {}

            def mm(sl, blk, kcs):
                def go():
                    if "q" not in st:
                        st["q"] = ps_ss.tile([128, 2, 512], F32, tag="ss",
                                             name="q2_ps")
                    for kc in kcs:
                        nc.tensor.matmul(
                            out=st["q"][:, sl, :],
                            lhsT=w_q[:, kc, :],
                            rhs=xbs[blk][:, kc, :],
                            start=(kc == 0),
                            stop=(kc == KC - 1),
                        )
                return go

            def q_adds():
                for sl, blk in ((0, blkA), (1, blkB)):
                    t0 = blk * 512
                    nc.vector.tensor_scalar_add(
                        out=qT[:, t0 : t0 + 512],
                        in0=st["q"][:, sl, :],
                        scalar1=b_q,
                    )

            return [
                mm(0, blkA, range(0, 4)),
                mm(0, blkA, range(4, 8)),
                mm(1, blkB, range(0, 4)),
                mm(1, blkB, range(4, 8)),
                q_adds,
            ]

        # ============== attention unit: (b, 512 queries, both heads) =======
        def attention_unit(b, u, fill, fill_rate2=0):
            """fill: list of closures to emit between jc steps (PE filler
            work for the scalar-bound stretch: late QKV blocks, early
            projections). fill_rate2: consume two fills per step for this
            many initial steps (front-loads producer work whose consumers
            are emitted later in the same unit)."""
            i0 = b * N + u * QBLK
            is_dbg = debug and b == 0 and u == 0
            if is_dbg:
                nc.sync.dma_start(out=dbg["dbg_qT"][:], in_=cast(qT[:, 0:512], attn_dt))
                nc.sync.dma_start(out=dbg["dbg_kT"][:], in_=cast(kT[:, 0:512], attn_dt))
                nc.sync.dma_start(out=dbg["dbg_v2ones"][:], in_=cast(V2[:, :, 64], attn_dt))
                nc.sync.dma_start(out=dbg["dbg_v2data"][:], in_=cast(V2[:, 0, :], attn_dt))
            poA = ps_po.tile([65, QBLK], F32, tag="po")
            poB = ps_po.tile([65, QBLK], F32, tag="po")

            def s_step(jc):
                j0 = b * N + jc * 128
                ss = ps_ss.tile([128, 2, QBLK], F32, tag="ss")
                for hl in range(2):
                    hs = hl * 64
                    nc.tensor.matmul(
                        out=ss[:, hl, :],
                        lhsT=kT[hs : hs + 64, j0 : j0 + 128],
                        rhs=qT[hs : hs + 64, i0 : i0 + QBLK],
                        start=True,
                        stop=True,
                    )
                pt = ptpool.tile([128, 2, QBLK], attn_dt, tag="pt")
                nc.scalar.activation(out=pt, in_=ss, func=EXP, scale=float(SCALE))
                return pt

            # software pipeline: S^T runs TWO key chunks ahead of PV so the
            # exp stream absorbs fill-work jitter without starving
            pts = [s_step(0), s_step(1)]
            if is_dbg:
                nc.sync.dma_start(out=dbg["dbg_pt0"][:], in_=cast(pts[0], attn_dt))
            fi = 0
            for jc in range(NJC):
                if jc + 2 < NJC:
                    pts.append(s_step(jc + 2))
                for _ in range(2 if jc < fill_rate2 else 1):
                    if fi < len(fill):
                        fill[fi]()
                        fi += 1
                for hl, po in ((0, poA), (1, poB)):
                    nc.tensor.matmul(
                        out=po,
                        lhsT=V2[:, b * (N // 128) + jc, hl * 65 : hl * 65 + 65],
                        rhs=pts[jc][:, hl, :],
                        start=(jc == 0),
                        stop=(jc == NJC - 1),
                    )
            for f in fill[fi:]:
                f()
            if is_dbg:
                dstA = unpool.tile([65, 512], F32, tag="dbgA")
                dstB = unpool.tile([65, 512], F32, tag="dbgB")
                nc.vector.tensor_copy(out=dstA, in_=poA)
                nc.vector.tensor_copy(out=dstB, in_=poB)
                nc.sync.dma_start(out=dbg["dbg_poA"][:], in_=dstA)
                nc.sync.dma_start(out=dbg["dbg_poB"][:], in_=dstB)

            # ---- evict part A (DVE-only; frees both po banks): pull the
            # unnormalized rows to SBUF and reciprocal the denominators ----
            unS = unpool.tile([64, 2, QBLK], F32, tag="un")
            rc = rcpool.tile([128, 2, QBLK], F32, tag="rc")
            rc2 = rcpool.tile([128, 2, QBLK], attn_dt, tag="rc2")
            dn = rcpool.tile([128, 2, QBLK], F32, tag="dn")
            for hl, po in ((0, poA), (1, poB)):
                nc.vector.tensor_copy(out=unS[:, hl, :], in_=po[0:64, :])
                nc.vector.tensor_copy(out=dn[64:65, hl, :], in_=po[64:65, :])
            # custom DVE ops require base_partition 0: run the reciprocal
            # over the full partition range (same per-lane cost; only row
            # 64 is meaningful downstream)
            nc.vector.reciprocal_approx_fast(out=rc, in_=dn)
            # round the fp32 reciprocals to the matmul dtype (verifier
            # requires f32r matmul operands to be f32r-rounded at source)
            nc.vector.tensor_copy(out=rc2[64:65, :, :], in_=rc[64:65, :, :])
            if is_dbg:
                nc.sync.dma_start(out=dbg["dbg_rc"][:], in_=rc[64:65, :, :])

            # ---- evict part B (has PE work: deferred into the next unit's
            # stream so it never delays the S->exp cadence) ----
            def evict_b():
                bct = ps_ss.tile([128, 2, QBLK], F32, tag="ss", name="bc_ps")
                for hl in range(2):
                    # [64,512] = 1/denom broadcast via ones outer product
                    nc.tensor.matmul(
                        out=bct[0:64, hl, :],
                        lhsT=onesr[64:65, :],
                        rhs=rc2[64:65, hl, :],
                        start=True,
                        stop=True,
                    )
                if is_dbg:
                    dstC = unpool.tile([64, 2, 512], F32, tag="dbgC")
                    nc.vector.tensor_copy(out=dstC, in_=bct[0:64, :, :])
                    nc.sync.dma_start(out=dbg["dbg_bct"][:], in_=dstC)
                unr = nrpool.tile([64, 2, QBLK], proj_dt, tag="unr")
                for hl in range(2):
                    nc.vector.tensor_mul(
                        out=unr[:, hl, :],
                        in0=unS[:, hl, :],
                        in1=bct[0:64, hl, :],
                    )
                # both heads staged in ONE strided DMA (dram row hl*64+r
                # <- unr[r, hl, :])
                if b == 0:
                    tgt = ag_in0[u // 2].rearrange("(h r) t -> r h t", h=2)[
                        :, :, (u % 2) * QBLK : (u % 2 + 1) * QBLK
                    ]
                else:
                    tgt = ag_in1[u].rearrange("(h r) t -> r h t", h=2)
                nc.gpsimd.dma_start(out=tgt, in_=unr)
                if is_dbg:
                    nc.sync.dma_start(out=dbg["dbg_unr"][:], in_=unr)
                if with_collective:
                    if b == 0 and u % 2 == 1:
                        nc.gpsimd.collective_compute(
                            "AllGather",
                            mybir.AluOpType.bypass,
                            ins=[ag_in0[u // 2][:]],
                            outs=[ag_out0[u // 2][:]],
                            replica_groups=[list(range(NCORES))],
                        )
                    elif b == 1:
                        nc.gpsimd.collective_compute(
                            "AllGather",
                            mybir.AluOpType.bypass,
                            ins=[ag_in1[u][:]],
                            outs=[ag_out1[u][:]],
                            replica_groups=[list(range(NCORES))],
                        )

            return evict_b

        # ========= output projection for a pair of 512-token units =========
        def proj0_dma(h):
            # b=0 readback: one [D, 1024] gather split across two queues
            ag_r = ag_out0[h].rearrange("(kc p) t -> p kc t", p=128)
            ab = apool.tile([128, KC, 1024], proj_dt, tag="ab0", bufs=2)
            nc.sync.dma_start(out=ab[:, 0:4, :], in_=ag_r[:, 0:4, :])
            nc.gpsimd.dma_start(out=ab[:, 4:8, :], in_=ag_r[:, 4:8, :])
            return ab

        def proj1_dma(u, engines=("sync",)):
            ag_r = ag_out1[u].rearrange("(kc p) t -> p kc t", p=128)
            ab = apool.tile([128, KC, 512], proj_dt, tag="ab")
            # a single DMA queue moves ~45GB/s: split across the given
            # queues. Only use queues that are idle (or whose pending work
            # can't be blocked) at the point this is EMITTED - a trigger
            # with an unmet dep head-of-line blocks its whole queue.
            n = len(engines)
            per = KC // n
            for i, e in enumerate(engines):
                eng = {"sync": nc.sync, "gpsimd": nc.gpsimd, "scalar": nc.scalar}[e]
                eng.dma_start(
                    out=ab[:, i * per : (i + 1) * per, :],
                    in_=ag_r[:, i * per : (i + 1) * per, :],
                )
            return ab

        def proj_fills(b, up, get_ab, store="sync"):
            st = {}

            def mm_group(j, kcs):
                def go():
                    if "ps" not in st:
                        st["ps"] = ps_ss.tile([128, 2, 512], F32, tag="ss", name="proj_ps")
                    for kc in kcs:
                        nc.tensor.matmul(
                            out=st["ps"][:, j, :],
                            lhsT=w_p[:, kc, :],
                            rhs=get_ab(j, kc),
                            start=(kc == 0),
                            stop=(kc == KC - 1),
                        )
                return go

            def evict():
                ot = oupool.tile([128, 2, 512], F32, tag="ot")
                nc.vector.tensor_scalar_add(out=ot, in0=st["ps"], scalar1=b_p)
                t0 = b * N + up * 1024
                if store == "split":
                    # final store: two idle queues in parallel
                    nc.scalar.dma_start(
                        out=out[:, t0 : t0 + 512], in_=ot[:, 0, :]
                    )
                    nc.gpsimd.dma_start(
                        out=out[:, t0 + 512 : t0 + 1024], in_=ot[:, 1, :]
                    )
                else:
                    eng = nc.scalar if store == "scalar" else nc.sync
                    eng.dma_start(out=out[:, t0 : t0 + 1024], in_=ot)

            return [
                mm_group(0, range(0, 4)),
                mm_group(0, range(4, 8)),
                mm_group(1, range(0, 4)),
                mm_group(1, range(4, 8)),
                evict,
            ]

        def ab0_get(abt, h):
            return lambda j, kc: abt[("b0", h)][:, kc, j * 512 : (j + 1) * 512]

        def ab1_get(abt, up):
            return lambda j, kc: abt[(1, up * 2 + j)][:, kc, :]

        def proj_compute(b, up, get_ab, store="sync"):
            for f in proj_fills(b, up, get_ab, store=store):
                f()

        # =========================== schedule ==============================
        # QKV for b=0 plus block 4 up front; the remaining b=1 QKV blocks
        # are spliced into b=0's scalar-bound attention as fine-grained
        # (~1-2us) closures so the S->exp feed is never starved. Each
        # unit's evict-with-PE-work (evict_b) is deferred into the NEXT
        # unit's stream for the same reason. The b=0 output projections
        # ride inside b=1's attention; b=1 projections run in the tail,
        # overlapped with the last AllGathers. All ag_out readbacks are
        # emitted on the sync queue BEFORE any out-store so a 0.5MB store
        # never delays a tail-critical load.
        # Emission order IS dependency order: a unit's S/PV instructions
        # may only consume qT/kT/V2 ranges whose producers were already
        # emitted. So: b=0's keys (k/v of blocks 0-3) run up front; the
        # q's of blocks 2-3 and everything for b=1 stream in as fill work,
        # always at least one unit ahead of their consumers.
        xbs = {}
        abt = {}
        # x block 0 first on all three queues, then w_k/w_v (they gate
        # only the k/v matmul groups, which run a few us after q)
        xbs[0] = qkv_dma(0)
        nc.gpsimd.dma_start(
            out=w_k, in_=cast(wk.rearrange("(kc p) m -> p kc m", p=128), qkv_dt)
        )
        nc.scalar.dma_start(
            out=w_v, in_=cast(wv.rearrange("(kc p) m -> p kc m", p=128), qkv_dt)
        )
        xbs[1] = qkv_dma(1)
        for blk in range(2):
            for f in qkv_fills(blk, xbs):
                f()
        for blk in (2, 3):
            xbs[blk] = qkv_dma(blk)
        xbs[4] = qkv_dma(4)
        xbs[5] = qkv_dma(5)
        # w_p and b_p are only needed in phase D; issue them after the
        # startup-critical x loads so they don't delay the first exps.
        nc.scalar.dma_start(out=b_p, in_=bp[:])
        nc.scalar.dma_start(
            out=w_p, in_=cast(wp.rearrange("(kc p) m -> p kc m", p=128), proj_dt)
        )
        # kv of blocks 2/3 ride INSIDE unit (0,0) at double fill rate: all
        # their closures are emitted before the S/PV instructions (emitted
        # at step jc for key-chunk jc+2) that consume them.
        b0_fillsets = [
            (lambda: kv_fills(2, xbs) + kv_fills(3, xbs)
             + [lambda: xbs.__setitem__(6, qkv_dma(6))]),
            (lambda: [lambda: xbs.__setitem__(7, qkv_dma(7))]
             + q2_fills(2, 3, xbs) + kv_fills(4, xbs)),
            (lambda: q2_fills(4, 5, xbs) + kv_fills(5, xbs)),
            (lambda: kv_fills(6, xbs) + kv_fills(7, xbs)),
        ]
        prev = None
        for u in range(NU):
            fill = ([prev] if prev else []) + b0_fillsets[u]()
            prev = attention_unit(0, u, fill, fill_rate2=(7 if u == 0 else 0))
        # b=1 units run in order 0,2,3,1 so the LAST unit's gather is the
        # only one outstanding at the end; the (1,2)/(1,3) readbacks and
        # most projection work complete inside unit (1,1)'s stream.
        nop = lambda: None
        p11 = proj_fills(1, 1, ab1_get(abt, 1), store="scalar")
        for seq, u in enumerate([0, 2, 3, 1]):
            fill = [prev]
            if seq == 0:
                fill += q2_fills(6, 7, xbs)
                fill += [lambda: abt.__setitem__(("b0", 0), proj0_dma(0))]
            elif seq == 1:
                fill += [lambda: abt.__setitem__(("b0", 1), proj0_dma(1))]
                fill += proj_fills(0, 0, ab0_get(abt, 0))
            elif seq == 2:
                # ab(1,0)'s AG completed during the first b1 unit: sync-only
                # (its dep is met before the queue reaches it => no block)
                fill += [
                    lambda: abt.__setitem__((1, 0), proj1_dma(0, ("sync",))),
                ] + proj_fills(0, 1, ab0_get(abt, 1))
            else:

                def ab13_first_half():
                    ab = apool.tile([128, KC, 512], proj_dt, tag="ab",
                                    name="ab13")
                    ag_r = ag_out1[3].rearrange("(kc p) t -> p kc t", p=128)
                    nc.sync.dma_start(
                        out=ab[:, 0:4, :], in_=ag_r[:, 0:4, :]
                    )
                    abt[(1, 3)] = ab

                fill += [
                    lambda: abt.__setitem__(
                        (1, 2), proj1_dma(2, ("sync", "gpsimd"))
                    ),
                    ab13_first_half,
                    nop,
                    nop,
                    p11[0],
                    p11[1],
                ]
            prev = attention_unit(1, u, fill)
        # ---------------- tail: exps are done, all queues free -------------
        prev()  # evict_b of unit (1,1) -> fires the final AllGather FIRST
        # second half of ab(1,3) on gpsimd, safely behind the AG trigger
        ag_r13 = ag_out1[3].rearrange("(kc p) t -> p kc t", p=128)
        nc.gpsimd.dma_start(out=abt[(1, 3)][:, 4:8, :], in_=ag_r13[:, 4:8, :])
        p11[2]()
        p11[3]()  # chunk (1,3): its readback completes ~10us into the tail
        p11[4]()  # pair (1,1) bias-add + store
        p10 = proj_fills(1, 0, ab1_get(abt, 0), store="split")
        p10[0]()
        p10[1]()  # chunk (1,0): its readback landed two units ago
        abt[(1, 1)] = proj1_dma(1, ("sync", "gpsimd"))
        p10[2]()
        p10[3]()
        p10[4]()

        rep_ctx.close()

    nc.compile()
    return nc


def np_dt(dt):
    return mybir.dt.np(F32 if dt == F32R else dt)


def prep_in_maps(x, Wqkv, bqkv, Wproj, bproj, qkv_dt=BF16, attn_dt=F32R,
                 proj_dt=BF16):
    x = np.asarray(x, dtype=np.float32)
    Wqkv = np.asarray(Wqkv, dtype=np.float32)
    bqkv = np.asarray(bqkv, dtype=np.float32)
    Wproj = np.asarray(Wproj, dtype=np.float32)
    bproj = np.asarray(bproj, dtype=np.float32)

    xtn = np.ascontiguousarray(x.reshape(TOK, D).T).astype(np_dt(qkv_dt))
    identity = np.eye(128, dtype=np_dt(attn_dt))

    # AllGather output rows are rank-major: row c*128 + hl*64 + d holds
    # feature (2c+hl)*64 + d; permute Wproj's contraction rows to match.
    wp_row_perm = np.empty(D, dtype=np.int64)
    for cc in range(NCORES):
        for hlhl in range(2):
            rows = np.arange(64)
            wp_row_perm[cc * 128 + hlhl * 64 + rows] = (2 * cc + hlhl) * 64 + rows

    # qkv column index for (head h, depth d, which): h*192 + d*3 + which
    d_idx = np.arange(DEPTH)
    in_maps = []
    for c in range(NCORES):
        h0, h1 = 2 * c, 2 * c + 1
        qcols = np.concatenate([h0 * 192 + 3 * d_idx, h1 * 192 + 3 * d_idx])
        kcols = qcols + 1
        vcols = qcols + 2
        in_maps.append(
            {
                "xt": xtn,
                "wq": np.ascontiguousarray(Wqkv[:, qcols]).astype(np_dt(qkv_dt)),
                "wk": np.ascontiguousarray(Wqkv[:, kcols]).astype(np_dt(qkv_dt)),
                "wv": np.ascontiguousarray(Wqkv[:, vcols]).astype(np_dt(qkv_dt)),
                "wp": np.ascontiguousarray(
                    Wproj[wp_row_perm, 128 * c : 128 * (c + 1)]
                ).astype(np_dt(proj_dt)),
                "bq": np.ascontiguousarray(bqkv[qcols]).reshape(128, 1),
                "bk": np.ascontiguousarray(bqkv[kcols]).reshape(128, 1),
                "bv": np.ascontiguousarray(bqkv[vcols]).reshape(128, 1),
                "bp": np.ascontiguousarray(
                    bproj[128 * c : 128 * (c + 1)]
                ).reshape(128, 1),
                "ident": identity,
            }
        )
    return in_maps


def assemble(results):
    outT = np.concatenate([r["o"] for r in results], axis=0)  # [D, TOK]
    return np.ascontiguousarray(outT.T).reshape(B, N, D).astype(np.float32)


# Chosen dtype config (see build_nc docstring for the tradeoff).
CONFIG = {"qkv_dt": BF16, "attn_dt": F16, "proj_dt": BF16}

_NC_CACHE = {}


def get_nc():
    if "nc" not in _NC_CACHE:
        _NC_CACHE["nc"] = build_nc(**CONFIG)
    return _NC_CACHE["nc"]


def kernel(x, Wqkv, bqkv, Wproj, bproj):
    nc = get_nc()
    in_maps = prep_in_maps(x, Wqkv, bqkv, Wproj, bproj, **CONFIG)
    res = run_bass_kernel_spmd(nc, in_maps, list(range(NCORES)))
    return assemble(res.results)


# revision 72
# speedup vs baseline: 1.6563x; 1.1294x over previous
"""Multi-head attention TRN2 kernel (B=2, N=2048, D=1024, H=16).

Sharding: tensor-parallel over heads. Each of the 8 cores owns 2 heads
(both batch elements) end-to-end through QKV projection and attention,
then per-head attention outputs are AllGathered in 512-token chunks and
each core computes a 128-column slice of the output projection.

Key performance structure (vs the v1 kernel):
- S^T matmuls contract over depth=64 only, so the two heads' S matmuls
  are issued back-to-back: they land on disjoint PE row-groups (rows
  0-63 vs 64-127 via base-partition-derived tile_position) and execute
  CONCURRENTLY, halving S cost. The S^T stream runs two key chunks
  ahead of exp so fill work never starves the scalar engine.
- exp() runs on [128, 2, 512] score tiles (both heads of one key chunk)
  to amortize the ~352-cycle fixed ACTIVATE cost; the scalar engine
  (~1.15us per exp, ~147us total) and the PE are closely balanced.
- Softmax normalization: the denominator row (from a ones-column in the
  PV matmul) is reciprocal'd with the fast approx DVE op (base
  partition 0 only!), broadcast across 64 partitions with a tiny PE
  matmul (ones outer product), and multiplied in - no 1-partition
  iterative reciprocal, no DRAM broadcast roundtrip. The PE part of
  each evict is deferred into the next unit's stream.
- Collectives: the CC stream serializes, so b=0 uses two 1024-token
  AllGathers (low stream occupancy) and b=1 four 512-token ones (small
  tail). b=1 units run in order 0,2,3,1 so only the final unit's
  gather is outstanding at the end.
- Almost all QKV work and the output projections ride as fine-grained
  (~1-2us) fill closures inside the attention units' jc loops, always
  emitted before their consumers (emission order IS dependency order).
- DMA: a single queue moves only ~45GB/s, so startup x-blocks split
  3 ways (sync/gpsimd/scalar) and readbacks 2 ways. AllGather-dependent
  DMA triggers never go on the scalar queue: the Tile scheduler may
  hoist them into the exp stream where their wait blocks the engine.
- dtypes: x/Wqkv/Wproj/collective traffic bf16 (halves DMA + fast
  weight load); attention operands fp16 (1cyc/row like bf16 but 10
  mantissa bits, and FWL-capable unlike f32r); all psum accumulation
  fp32. Scaled absmax error ~4e-3.

Self-contained: hardcodes shapes from the problem spec.
"""

import sys

for _p in ("/opt/trn_rl_repo", "/root/.axon_site/_ro/trn_rl_repo"):
    if _p not in sys.path:
        sys.path.append(_p)

import numpy as np
from contextlib import ExitStack

import concourse.bass as bass
import concourse.tile as tile
from concourse import mybir, bacc
from concourse.bass_utils import run_bass_kernel_spmd

F32 = mybir.dt.float32
F32R = mybir.dt.float32r
BF16 = mybir.dt.bfloat16
F16 = mybir.dt.float16
EXP = mybir.ActivationFunctionType.Exp

B = 2
N = 2048
D = 1024
H = 16
DEPTH = 64
TOK = B * N            # 4096 tokens total (both batches)
KC = D // 128          # 8 contraction chunks of 128
NBLK = TOK // 512      # 8 token blocks for streaming projections
SCALE = 1.0 / np.sqrt(DEPTH)
NCORES = 8
QBLK = 512             # query-block width in attention (= AllGather grain)
NU = N // QBLK         # 4 attention units per batch element
NJC = N // 128         # 16 key chunks per unit


def build_nc(reps=1, with_collective=True, qkv_dt=BF16, attn_dt=F32R,
             proj_dt=BF16, debug=False):
    """Build the per-core kernel program.

    reps>1 wraps the compute in a For_i hardware loop for benchmarking
    (collectives are skipped: they cannot appear inside control flow).

    qkv_dt: dtype of x^T and QKV weights (the QKV matmuls).
    attn_dt: dtype of q^T/k^T/V2/P^T (the S^T and P^T@V matmuls).
    proj_dt: dtype of the AllGathered A^T and Wproj (projection matmuls).
    """
    bench = reps > 1
    nc = bacc.Bacc(None)

    def dram_dt(dt):
        return F32 if dt == F32R else dt

    def cast(ap, dt):
        return ap.bitcast(F32R) if dt == F32R else ap

    xt = nc.dram_tensor("xt", [D, TOK], dram_dt(qkv_dt), kind="ExternalInput")
    wq = nc.dram_tensor("wq", [D, 128], dram_dt(qkv_dt), kind="ExternalInput")
    wk = nc.dram_tensor("wk", [D, 128], dram_dt(qkv_dt), kind="ExternalInput")
    wv = nc.dram_tensor("wv", [D, 128], dram_dt(qkv_dt), kind="ExternalInput")
    wp = nc.dram_tensor("wp", [D, 128], dram_dt(proj_dt), kind="ExternalInput")
    bq = nc.dram_tensor("bq", [128, 1], F32, kind="ExternalInput")
    bk = nc.dram_tensor("bk", [128, 1], F32, kind="ExternalInput")
    bv = nc.dram_tensor("bv", [128, 1], F32, kind="ExternalInput")
    bp = nc.dram_tensor("bp", [128, 1], F32, kind="ExternalInput")
    ident = nc.dram_tensor(
        "ident", [128, 128], dram_dt(attn_dt), kind="ExternalInput"
    )
    out = nc.dram_tensor("o", [128, TOK], F32, kind="ExternalOutput")

    # Collective staging. The CC stream serializes all collectives, so b=0
    # (whose gathers have slack) uses two 1024-token AllGathers while b=1
    # (whose last gather is tail-critical) uses four 512-token ones - this
    # keeps total stream occupancy low AND the final gather small.
    ag_in0 = [nc.dram_tensor(f"agi0_{h}", [128, 2 * QBLK], proj_dt)
              for h in range(2)]
    ag_out0 = [
        nc.dram_tensor(f"ago0_{h}", [D, 2 * QBLK], proj_dt, addr_space="Shared")
        for h in range(2)
    ]
    ag_in1 = [nc.dram_tensor(f"agi1_{u}", [128, QBLK], proj_dt)
              for u in range(NU)]
    ag_out1 = [
        nc.dram_tensor(f"ago1_{u}", [D, QBLK], proj_dt, addr_space="Shared")
        for u in range(NU)
    ]

    dbg = {}
    if debug:
        for name, shape, dt in [
            ("dbg_qT", [128, 512], dram_dt(attn_dt)),
            ("dbg_kT", [128, 512], dram_dt(attn_dt)),
            ("dbg_v2ones", [128, 32], dram_dt(attn_dt)),
            ("dbg_v2data", [128, 130], dram_dt(attn_dt)),
            ("dbg_pt0", [128, 2, 512], dram_dt(attn_dt)),
            ("dbg_poA", [65, 512], F32),
            ("dbg_poB", [65, 512], F32),
            ("dbg_rc", [1, 2, 512], F32),
            ("dbg_bct", [64, 1024], F32),
            ("dbg_unr", [64, 2, 512], dram_dt(proj_dt)),
        ]:
            dbg[name] = nc.dram_tensor(name, shape, dt, kind="ExternalOutput")

    xt_r = xt.rearrange("(kc p) t -> p kc t", p=128)

    with tile.TileContext(nc) as tc, ExitStack() as ctx:
        wpool = ctx.enter_context(tc.tile_pool(name="w", bufs=1))
        qkpool = ctx.enter_context(tc.tile_pool(name="qk", bufs=1))
        vpool = ctx.enter_context(tc.tile_pool(name="v2", bufs=1))
        xpool = ctx.enter_context(tc.tile_pool(name="x", bufs=5))
        vtpool = ctx.enter_context(tc.tile_pool(name="vt", bufs=2))
        ptpool = ctx.enter_context(tc.tile_pool(name="pt", bufs=4))
        unpool = ctx.enter_context(tc.tile_pool(name="un", bufs=2))
        rcpool = ctx.enter_context(tc.tile_pool(name="rc", bufs=2))
        nrpool = ctx.enter_context(tc.tile_pool(name="nr", bufs=2))
        apool = ctx.enter_context(tc.tile_pool(name="ap", bufs=4))
        oupool = ctx.enter_context(tc.tile_pool(name="ou", bufs=2))
        # PSUM budget (8 banks of 2KB/partition):
        #   ps_ss (bufs=3): [128,2,512] f32 (2 banks each)        -> 6 banks
        #     (QKV q|k pairs, score pairs, proj chunk pairs, v-transpose
        #     staging, the denominator broadcast) - 3 bufs let the S^T
        #     stream run two key-chunks ahead of exp, so fill work never
        #     starves the scalar engine
        #   ps_po (bufs=2): [65,512] f32 po accumulators          -> 2 banks
        ps_ss = ctx.enter_context(tc.tile_pool(name="pss", bufs=3, space="PSUM"))
        ps_po = ctx.enter_context(tc.tile_pool(name="psp", bufs=2, space="PSUM"))

        # ---- weights / constants (loaded once, outside any rep loop) ----
        # Order matters: w_q + the first x block gate the first matmul.
        # w_q is split across two queues so its first half (all the first
        # q matmul group needs) lands in ~3us; w_k/w_v follow the x thirds
        # on their queues since they gate only the later k/v groups.
        w_q = wpool.tile([128, KC, 128], qkv_dt, tag="w_q")
        w_k = wpool.tile([128, KC, 128], qkv_dt, tag="w_k")
        w_v = wpool.tile([128, KC, 128], qkv_dt, tag="w_v")
        w_p = wpool.tile([128, KC, 128], proj_dt, tag="w_p")
        wq_r = wq.rearrange("(kc p) m -> p kc m", p=128)
        nc.sync.dma_start(out=w_q[:, 0:4, :], in_=cast(wq_r[:, 0:4, :], qkv_dt))
        nc.gpsimd.dma_start(out=w_q[:, 4:8, :], in_=cast(wq_r[:, 4:8, :], qkv_dt))
        b_q = wpool.tile([128, 1], F32, tag="b_q")
        b_k = wpool.tile([128, 1], F32, tag="b_k")
        b_v = wpool.tile([128, 1], F32, tag="b_v")
        b_p = wpool.tile([128, 1], F32, tag="b_p")
        for t, src in ((b_q, bq), (b_k, bk), (b_v, bv)):
            nc.scalar.dma_start(out=t, in_=src[:])
        id_t = wpool.tile([128, 128], attn_dt, tag="id_t")
        nc.scalar.dma_start(out=id_t, in_=cast(ident[:], attn_dt))

        # qT/kT: [feature 128 (= 2 heads x 64), token 4096]; head hl in rows
        # hl*64:(hl+1)*64 so both S^T operands share a partition base.
        qT = qkpool.tile([128, TOK], attn_dt, tag="qT")
        kT = qkpool.tile([128, TOK], attn_dt, tag="kT")
        # V2: [token part, 32 token-chunks, 130]: v_h0 | ones | v_h1 | ones
        V2 = vpool.tile([128, TOK // 128, 130], attn_dt, tag="V2")
        # memset can't emit f32r; stage fp32 ones and round via tensor_copy
        onesF = wpool.tile([128, 64], F32, tag="onesF")
        nc.vector.memset(onesF, 1.0)
        nc.vector.tensor_copy(out=V2[:, :, 64], in_=onesF[:, 0:32])
        nc.vector.tensor_copy(out=V2[:, :, 129], in_=onesF[:, 0:32])
        # ones row at partition 64 for the denominator broadcast matmuls
        onesr = wpool.tile([128, 64], attn_dt, tag="onesr")
        nc.vector.tensor_copy(out=onesr[64:65, :], in_=onesF[64:65, :])


        rep_ctx = ExitStack()
        if bench:
            rep_ctx.enter_context(
                tc.For_i(
                    0,
                    reps,
                    1,
                    hint_engines=(
                        mybir.EngineType.PE,
                        mybir.EngineType.Activation,
                        mybir.EngineType.DVE,
                        mybir.EngineType.SP,
                    ),
                )
            )

        # ================= QKV projection for one 512-token block ==========
        def qkv_dma(blk):
            t0 = blk * 512
            xb = xpool.tile([128, KC, 512], qkv_dt, tag="xb")
            # a single DMA queue moves only ~45GB/s: split across queues.
            # The upfront b=0 blocks are DMA-critical: 3-way split (the
            # scalar queue is free before the exps start). Later blocks
            # are prefetched with a full unit of lead: 2-way is enough.
            if blk < 4:
                nc.sync.dma_start(
                    out=xb[:, 0:3, :], in_=cast(xt_r[:, 0:3, t0 : t0 + 512], qkv_dt)
                )
                nc.gpsimd.dma_start(
                    out=xb[:, 3:6, :], in_=cast(xt_r[:, 3:6, t0 : t0 + 512], qkv_dt)
                )
                nc.scalar.dma_start(
                    out=xb[:, 6:8, :], in_=cast(xt_r[:, 6:8, t0 : t0 + 512], qkv_dt)
                )
            else:
                nc.sync.dma_start(
                    out=xb[:, 0:4, :], in_=cast(xt_r[:, 0:4, t0 : t0 + 512], qkv_dt)
                )
                nc.gpsimd.dma_start(
                    out=xb[:, 4:8, :], in_=cast(xt_r[:, 4:8, t0 : t0 + 512], qkv_dt)
                )
            return xb

        def qkv_fills(blk, xbs, st=None):
            """Return fine-grained closures (~1-2us of PE work each) that
            together compute QKV for one block; for splicing into the
            scalar-bound attention stream without stalling the exp feed."""
            t0 = blk * 512
            st = {} if st is None else st

            def mm_group(pskey, sl, w_t, kcs):
                def go():
                    if pskey not in st:
                        st[pskey] = ps_ss.tile([128, 2, 512], F32, tag="ss", name=f"qkv_{pskey}")
                    for kc in kcs:
                        nc.tensor.matmul(
                            out=st[pskey][:, sl, :],
                            lhsT=w_t[:, kc, :],
                            rhs=xbs[blk][:, kc, :],
                            start=(kc == 0),
                            stop=(kc == KC - 1),
                        )
                return go

            def qk_adds():
                nc.vector.tensor_scalar_add(
                    out=qT[:, t0 : t0 + 512], in0=st["qk"][:, 0, :], scalar1=b_q
                )
                nc.vector.tensor_scalar_add(
                    out=kT[:, t0 : t0 + 512], in0=st["qk"][:, 1, :], scalar1=b_k
                )

            def v_trans():
                vtmp = vtpool.tile([128, 512], attn_dt, tag="vtmp")
                nc.vector.tensor_scalar_add(
                    out=vtmp, in0=st["v"][:, 0, :], scalar1=b_v
                )
                # transpose back into slot 1 of the same psum tile (vtmp
                # holds the slot-0 data already): avoids a third ss-pool
                # allocation per block, letting consecutive QKV blocks
                # pipeline within 3 bufs
                wd = 128 if attn_dt == F32R else 64  # f32-cols per transpose
                for s in range(4):
                    nc.tensor.transpose(
                        out=st["v"][:, 1, s * wd : (s + 1) * wd].bitcast(attn_dt),
                        in_=vtmp[:, s * 128 : (s + 1) * 128],
                        identity=id_t,
                    )

            def v_copies():
                wd = 128 if attn_dt == F32R else 64
                for s in range(4):
                    ch = blk * 4 + s
                    src = st["v"][:, 1, s * wd : (s + 1) * wd].bitcast(attn_dt)
                    nc.vector.tensor_copy(out=V2[:, ch, 0:64], in_=src[:, 0:64])
                    nc.vector.tensor_copy(
                        out=V2[:, ch, 65:129], in_=src[:, 64:128]
                    )

            return [
                mm_group("qk", 0, w_q, range(0, 4)),
                mm_group("qk", 0, w_q, range(4, 8)),
                mm_group("qk", 1, w_k, range(0, 4)),
                mm_group("qk", 1, w_k, range(4, 8)),
                qk_adds,
                mm_group("v", 0, w_v, range(0, 4)),
                mm_group("v", 0, w_v, range(4, 8)),
                v_trans,
                v_copies,
            ]

        def qkv_block(blk):
            xb = qkv_dma(blk)
            for f in qkv_fills(blk, {blk: xb}):
                f()

        # -- finer-grained variants: k+v (needed as attention keys one unit
        # ahead) separately from q (needed as queries ~two units later) --
        def kv_fills(blk, xbs):
            """k into slot 0 and v into slot 1 of one psum tile; the v
            transposes reuse slot 0 after the k bias-add has drained it."""
            t0 = blk * 512
            st = {}

            def mm(sl, w_t, kcs):
                def go():
                    if "kv" not in st:
                        st["kv"] = ps_ss.tile([128, 2, 512], F32, tag="ss",
                                              name="kv_ps")
                    for kc in kcs:
                        nc.tensor.matmul(
                            out=st["kv"][:, sl, :],
                            lhsT=w_t[:, kc, :],
                            rhs=xbs[blk][:, kc, :],
                            start=(kc == 0),
                            stop=(kc == KC - 1),
                        )
                return go

            def k_add():
                nc.vector.tensor_scalar_add(
                    out=kT[:, t0 : t0 + 512], in0=st["kv"][:, 0, :], scalar1=b_k
                )

            def v_trans():
                vtmp = vtpool.tile([128, 512], attn_dt, tag="vtmp")
                nc.vector.tensor_scalar_add(
                    out=vtmp, in0=st["kv"][:, 1, :], scalar1=b_v
                )
                wd = 128 if attn_dt == F32R else 64
                for s in range(4):
                    nc.tensor.transpose(
                        out=st["kv"][:, 0, s * wd : (s + 1) * wd].bitcast(attn_dt),
                        in_=vtmp[:, s * 128 : (s + 1) * 128],
                        identity=id_t,
                    )

            def v_copies():
                wd = 128 if attn_dt == F32R else 64
                for s in range(4):
                    ch = blk * 4 + s
                    src = st["kv"][:, 0, s * wd : (s + 1) * wd].bitcast(attn_dt)
                    nc.vector.tensor_copy(out=V2[:, ch, 0:64], in_=src[:, 0:64])
                    nc.vector.tensor_copy(
                        out=V2[:, ch, 65:129], in_=src[:, 64:128]
                    )

            return [
                mm(0, w_k, range(0, 4)),
                mm(0, w_k, range(4, 8)),
                k_add,
                mm(1, w_v, range(0, 4)),
                mm(1, w_v, range(4, 8)),
                v_trans,
                v_copies,
            ]

        def q2_fills(blkA, blkB, xbs):
            """q projections for two blocks sharing one psum tile."""
            st = {}

            def mm(sl, blk, kcs):
                def go():
                    if "q" not in st:
                        st["q"] = ps_ss.tile([128, 2, 512], F32, tag="ss",
                                             name="q2_ps")
                    for kc in kcs:
                        nc.tensor.matmul(
                            out=st["q"][:, sl, :],
                            lhsT=w_q[:, kc, :],
                            rhs=xbs[blk][:, kc, :],
                            start=(kc == 0),
                            stop=(kc == KC - 1),
                        )
                return go

            def q_adds():
                for sl, blk in ((0, blkA), (1, blkB)):
                    t0 = blk * 512
                    nc.vector.tensor_scalar_add(
                        out=qT[:, t0 : t0 + 512],
                        in0=st["q"][:, sl, :],
                        scalar1=b_q,
                    )

            return [
                mm(0, blkA, range(0, 4)),
                mm(0, blkA, range(4, 8)),
                mm(1, blkB, range(0, 4)),
                mm(1, blkB, range(4, 8)),
                q_adds,
            ]

        # ============== attention unit: (b, 512 queries, both heads) =======
        def attention_unit(b, u, fill, fill_rate2=0):
            """fill: list of closures to emit between jc steps (PE filler
            work for the scalar-bound stretch: late QKV blocks, early
            projections). fill_rate2: consume two fills per step for this
            many initial steps (front-loads producer work whose consumers
            are emitted later in the same unit)."""
            i0 = b * N + u * QBLK
            is_dbg = debug and b == 0 and u == 0
            if is_dbg:
                nc.sync.dma_start(out=dbg["dbg_qT"][:], in_=cast(qT[:, 0:512], attn_dt))
                nc.sync.dma_start(out=dbg["dbg_kT"][:], in_=cast(kT[:, 0:512], attn_dt))
                nc.sync.dma_start(out=dbg["dbg_v2ones"][:], in_=cast(V2[:, :, 64], attn_dt))
                nc.sync.dma_start(out=dbg["dbg_v2data"][:], in_=cast(V2[:, 0, :], attn_dt))
            poA = ps_po.tile([65, QBLK], F32, tag="po")
            poB = ps_po.tile([65, QBLK], F32, tag="po")

            def s_step(jc):
                j0 = b * N + jc * 128
                ss = ps_ss.tile([128, 2, QBLK], F32, tag="ss")
                for hl in range(2):
                    hs = hl * 64
                    nc.tensor.matmul(
                        out=ss[:, hl, :],
                        lhsT=kT[hs : hs + 64, j0 : j0 + 128],
                        rhs=qT[hs : hs + 64, i0 : i0 + QBLK],
                        start=True,
                        stop=True,
                    )
                pt = ptpool.tile([128, 2, QBLK], attn_dt, tag="pt")
                nc.scalar.activation(out=pt, in_=ss, func=EXP, scale=float(SCALE))
                return pt

            # software pipeline: S^T runs TWO key chunks ahead of PV so the
            # exp stream absorbs fill-work jitter without starving
            pts = [s_step(0), s_step(1)]
            if is_dbg:
                nc.sync.dma_start(out=dbg["dbg_pt0"][:], in_=cast(pts[0], attn_dt))
            fi = 0
            for jc in range(NJC):
                if jc + 2 < NJC:
                    pts.append(s_step(jc + 2))
                for _ in range(2 if jc < fill_rate2 else 1):
                    if fi < len(fill):
                        fill[fi]()
                        fi += 1
                for hl, po in ((0, poA), (1, poB)):
                    nc.tensor.matmul(
                        out=po,
                        lhsT=V2[:, b * (N // 128) + jc, hl * 65 : hl * 65 + 65],
                        rhs=pts[jc][:, hl, :],
                        start=(jc == 0),
                        stop=(jc == NJC - 1),
                    )
            for f in fill[fi:]:
                f()
            if is_dbg:
                dstA = unpool.tile([65, 512], F32, tag="dbgA")
                dstB = unpool.tile([65, 512], F32, tag="dbgB")
                nc.vector.tensor_copy(out=dstA, in_=poA)
                nc.vector.tensor_copy(out=dstB, in_=poB)
                nc.sync.dma_start(out=dbg["dbg_poA"][:], in_=dstA)
                nc.sync.dma_start(out=dbg["dbg_poB"][:], in_=dstB)

            # ---- evict part A (DVE-only; frees both po banks): pull the
            # unnormalized rows to SBUF and reciprocal the denominators ----
            unS = unpool.tile([64, 2, QBLK], F32, tag="un")
            rc = rcpool.tile([128, 2, QBLK], F32, tag="rc")
            rc2 = rcpool.tile([128, 2, QBLK], attn_dt, tag="rc2")
            dn = rcpool.tile([128, 2, QBLK], F32, tag="dn")
            for hl, po in ((0, poA), (1, poB)):
                nc.vector.tensor_copy(out=unS[:, hl, :], in_=po[0:64, :])
                nc.vector.tensor_copy(out=dn[64:65, hl, :], in_=po[64:65, :])
            # custom DVE ops require base_partition 0: run the reciprocal
            # over the full partition range (same per-lane cost; only row
            # 64 is meaningful downstream)
            nc.vector.reciprocal_approx_fast(out=rc, in_=dn)
            # round the fp32 reciprocals to the matmul dtype (verifier
            # requires f32r matmul operands to be f32r-rounded at source)
            nc.vector.tensor_copy(out=rc2[64:65, :, :], in_=rc[64:65, :, :])
            if is_dbg:
                nc.sync.dma_start(out=dbg["dbg_rc"][:], in_=rc[64:65, :, :])

            # ---- evict part B (has PE work: deferred into the next unit's
            # stream so it never delays the S->exp cadence) ----
            def evict_b():
                bct = ps_ss.tile([128, 2, QBLK], F32, tag="ss", name="bc_ps")
                for hl in range(2):
                    # [64,512] = 1/denom broadcast via ones outer product
                    nc.tensor.matmul(
                        out=bct[0:64, hl, :],
                        lhsT=onesr[64:65, :],
                        rhs=rc2[64:65, hl, :],
                        start=True,
                        stop=True,
                    )
                if is_dbg:
                    dstC = unpool.tile([64, 2, 512], F32, tag="dbgC")
                    nc.vector.tensor_copy(out=dstC, in_=bct[0:64, :, :])
                    nc.sync.dma_start(out=dbg["dbg_bct"][:], in_=dstC)
                unr = nrpool.tile([64, 2, QBLK], proj_dt, tag="unr")
                for hl in range(2):
                    nc.vector.tensor_mul(
                        out=unr[:, hl, :],
                        in0=unS[:, hl, :],
                        in1=bct[0:64, hl, :],
                    )
                # both heads staged in ONE strided DMA (dram row hl*64+r
                # <- unr[r, hl, :])
                if b == 0:
                    tgt = ag_in0[u // 2].rearrange("(h r) t -> r h t", h=2)[
                        :, :, (u % 2) * QBLK : (u % 2 + 1) * QBLK
                    ]
                else:
                    tgt = ag_in1[u].rearrange("(h r) t -> r h t", h=2)
                nc.gpsimd.dma_start(out=tgt, in_=unr)
                if is_dbg:
                    nc.sync.dma_start(out=dbg["dbg_unr"][:], in_=unr)
                if with_collective:
                    if b == 0 and u % 2 == 1:
                        nc.gpsimd.collective_compute(
                            "AllGather",
                            mybir.AluOpType.bypass,
                            ins=[ag_in0[u // 2][:]],
                            outs=[ag_out0[u // 2][:]],
                            replica_groups=[list(range(NCORES))],
                        )
                    elif b == 1:
                        nc.gpsimd.collective_compute(
                            "AllGather",
                            mybir.AluOpType.bypass,
                            ins=[ag_in1[u][:]],
                            outs=[ag_out1[u][:]],
                            replica_groups=[list(range(NCORES))],
                        )

            return evict_b

        # ========= output projection for a pair of 512-token units =========
        def proj0_dma(h):
            # b=0 readback: sync-only (a gpsimd half here would HOL-block
            # the gather staging/triggers behind a late AllGather), with
            # two units of lead before its consumers
            ag_r = ag_out0[h].rearrange("(kc p) t -> p kc t", p=128)
            ab = apool.tile([128, KC, 1024], proj_dt, tag="ab0", bufs=2)
            nc.sync.dma_start(out=ab, in_=ag_r)
            return ab

        def proj1_dma(u, engines=("sync",)):
            ag_r = ag_out1[u].rearrange("(kc p) t -> p kc t", p=128)
            ab = apool.tile([128, KC, 512], proj_dt, tag="ab")
            # a single DMA queue moves ~45GB/s: split across the given
            # queues. Only use queues that are idle (or whose pending work
            # can't be blocked) at the point this is EMITTED - a trigger
            # with an unmet dep head-of-line blocks its whole queue.
            n = len(engines)
            per = KC // n
            for i, e in enumerate(engines):
                eng = {"sync": nc.sync, "gpsimd": nc.gpsimd, "scalar": nc.scalar}[e]
                eng.dma_start(
                    out=ab[:, i * per : (i + 1) * per, :],
                    in_=ag_r[:, i * per : (i + 1) * per, :],
                )
            return ab

        def proj_fills(b, up, get_ab, store="sync"):
            st = {}

            def mm_group(j, kcs):
                def go():
                    if "ps" not in st:
                        st["ps"] = ps_ss.tile([128, 2, 512], F32, tag="ss", name="proj_ps")
                    for kc in kcs:
                        nc.tensor.matmul(
                            out=st["ps"][:, j, :],
                            lhsT=w_p[:, kc, :],
                            rhs=get_ab(j, kc),
                            start=(kc == 0),
                            stop=(kc == KC - 1),
                        )
                return go

            def evict():
                ot = oupool.tile([128, 2, 512], F32, tag="ot")
                nc.vector.tensor_scalar_add(out=ot, in0=st["ps"], scalar1=b_p)
                t0 = b * N + up * 1024
                if store == "split":
                    # final store: two idle queues in parallel
                    nc.scalar.dma_start(
                        out=out[:, t0 : t0 + 512], in_=ot[:, 0, :]
                    )
                    nc.gpsimd.dma_start(
                        out=out[:, t0 + 512 : t0 + 1024], in_=ot[:, 1, :]
                    )
                else:
                    eng = nc.scalar if store == "scalar" else nc.sync
                    eng.dma_start(out=out[:, t0 : t0 + 1024], in_=ot)

            return [
                mm_group(0, range(0, 4)),
                mm_group(0, range(4, 8)),
                mm_group(1, range(0, 4)),
                mm_group(1, range(4, 8)),
                evict,
            ]

        def ab0_get(abt, h):
            return lambda j, kc: abt[("b0", h)][:, kc, j * 512 : (j + 1) * 512]

        def ab1_get(abt, up):
            return lambda j, kc: abt[(1, up * 2 + j)][:, kc, :]

        def proj_compute(b, up, get_ab, store="sync"):
            for f in proj_fills(b, up, get_ab, store=store):
                f()

        # =========================== schedule ==============================
        # QKV for b=0 plus block 4 up front; the remaining b=1 QKV blocks
        # are spliced into b=0's scalar-bound attention as fine-grained
        # (~1-2us) closures so the S->exp feed is never starved. Each
        # unit's evict-with-PE-work (evict_b) is deferred into the NEXT
        # unit's stream for the same reason. The b=0 output projections
        # ride inside b=1's attention; b=1 projections run in the tail,
        # overlapped with the last AllGathers. All ag_out readbacks are
        # emitted on the sync queue BEFORE any out-store so a 0.5MB store
        # never delays a tail-critical load.
        # Emission order IS dependency order: a unit's S/PV instructions
        # may only consume qT/kT/V2 ranges whose producers were already
        # emitted. So: b=0's keys (k/v of blocks 0-3) run up front; the
        # q's of blocks 2-3 and everything for b=1 stream in as fill work,
        # always at least one unit ahead of their consumers.
        xbs = {}
        abt = {}
        # x block 0 first on all three queues, then w_k/w_v (they gate
        # only the k/v matmul groups, which run a few us after q)
        xbs[0] = qkv_dma(0)
        nc.gpsimd.dma_start(
            out=w_k, in_=cast(wk.rearrange("(kc p) m -> p kc m", p=128), qkv_dt)
        )
        nc.scalar.dma_start(
            out=w_v, in_=cast(wv.rearrange("(kc p) m -> p kc m", p=128), qkv_dt)
        )
        xbs[1] = qkv_dma(1)
        for blk in range(2):
            for f in qkv_fills(blk, xbs):
                f()
        for blk in (2, 3):
            xbs[blk] = qkv_dma(blk)
        xbs[4] = qkv_dma(4)
        xbs[5] = qkv_dma(5)
        # w_p and b_p are only needed in phase D; issue them after the
        # startup-critical x loads so they don't delay the first exps.
        nc.scalar.dma_start(out=b_p, in_=bp[:])
        nc.scalar.dma_start(
            out=w_p, in_=cast(wp.rearrange("(kc p) m -> p kc m", p=128), proj_dt)
        )
        # kv of blocks 2/3 ride INSIDE unit (0,0) at double fill rate: all
        # their closures are emitted before the S/PV instructions (emitted
        # at step jc for key-chunk jc+2) that consume them.
        b0_fillsets = [
            (lambda: kv_fills(2, xbs) + kv_fills(3, xbs)
             + [lambda: xbs.__setitem__(6, qkv_dma(6))]),
            (lambda: [lambda: xbs.__setitem__(7, qkv_dma(7))]
             + q2_fills(2, 3, xbs) + kv_fills(4, xbs)),
            (lambda: q2_fills(4, 5, xbs) + kv_fills(5, xbs)),
            (lambda: kv_fills(6, xbs) + kv_fills(7, xbs)),
        ]
        prev = None
        for u in range(NU):
            fill = ([prev] if prev else []) + b0_fillsets[u]()
            prev = attention_unit(0, u, fill, fill_rate2=(7 if u == 0 else 0))
        # b=1 units run in order 0,2,3,1 so the LAST unit's gather is the
        # only one outstanding at the end; the (1,2)/(1,3) readbacks and
        # most projection work complete inside unit (1,1)'s stream.
        nop = lambda: None
        p11 = proj_fills(1, 1, ab1_get(abt, 1), store="scalar")
        for seq, u in enumerate([0, 2, 3, 1]):
            fill = [prev]
            if seq == 0:
                fill += q2_fills(6, 7, xbs)
                fill += [lambda: abt.__setitem__(("b0", 0), proj0_dma(0))]
            elif seq == 1:
                fill += [
                    lambda: abt.__setitem__(("b0", 1), proj0_dma(1)),
                    lambda: abt.__setitem__((1, 0), proj1_dma(0, ("sync",))),
                ]
            elif seq == 2:
                fill += proj_fills(0, 0, ab0_get(abt, 0))
            else:

                def ab13_first_half():
                    ab = apool.tile([128, KC, 512], proj_dt, tag="ab",
                                    name="ab13")
                    ag_r = ag_out1[3].rearrange("(kc p) t -> p kc t", p=128)
                    nc.sync.dma_start(
                        out=ab[:, 0:4, :], in_=ag_r[:, 0:4, :]
                    )
                    abt[(1, 3)] = ab

                fill += [
                    lambda: abt.__setitem__((1, 2), proj1_dma(2, ("sync",))),
                    ab13_first_half,
                ] + proj_fills(0, 1, ab0_get(abt, 1))
            prev = attention_unit(1, u, fill)
        # ---------------- tail: exps are done, all queues free -------------
        prev()  # evict_b of unit (1,1) -> fires the final AllGather FIRST
        # second half of ab(1,3) on gpsimd, safely behind the AG trigger
        ag_r13 = ag_out1[3].rearrange("(kc p) t -> p kc t", p=128)
        nc.gpsimd.dma_start(out=abt[(1, 3)][:, 4:8, :], in_=ag_r13[:, 4:8, :])
        p11[0]()
        p11[1]()  # chunk (1,2): its readback landed during the last unit
        p11[2]()
        p11[3]()  # chunk (1,3): its readback completes ~10us into the tail
        p11[4]()  # pair (1,1) bias-add + store
        p10 = proj_fills(1, 0, ab1_get(abt, 0), store="split")
        p10[0]()
        p10[1]()  # chunk (1,0): its readback landed two units ago
        abt[(1, 1)] = proj1_dma(1, ("sync", "gpsimd"))
        p10[2]()
        p10[3]()
        p10[4]()

        rep_ctx.close()

    nc.compile()
    return nc


def np_dt(dt):
    return mybir.dt.np(F32 if dt == F32R else dt)


def prep_in_maps(x, Wqkv, bqkv, Wproj, bproj, qkv_dt=BF16, attn_dt=F32R,
                 proj_dt=BF16):
    x = np.asarray(x, dtype=np.float32)
    Wqkv = np.asarray(Wqkv, dtype=np.float32)
    bqkv = np.asarray(bqkv, dtype=np.float32)
    Wproj = np.asarray(Wproj, dtype=np.float32)
    bproj = np.asarray(bproj, dtype=np.float32)

    xtn = np.ascontiguousarray(x.reshape(TOK, D).T).astype(np_dt(qkv_dt))
    identity = np.eye(128, dtype=np_dt(attn_dt))

    # AllGather output rows are rank-major: row c*128 + hl*64 + d holds
    # feature (2c+hl)*64 + d; permute Wproj's contraction rows to match.
    wp_row_perm = np.empty(D, dtype=np.int64)
    for cc in range(NCORES):
        for hlhl in range(2):
            rows = np.arange(64)
            wp_row_perm[cc * 128 + hlhl * 64 + rows] = (2 * cc + hlhl) * 64 + rows

    # qkv column index for (head h, depth d, which): h*192 + d*3 + which
    d_idx = np.arange(DEPTH)
    in_maps = []
    for c in range(NCORES):
        h0, h1 = 2 * c, 2 * c + 1
        qcols = np.concatenate([h0 * 192 + 3 * d_idx, h1 * 192 + 3 * d_idx])
        kcols = qcols + 1
        vcols = qcols + 2
        in_maps.append(
            {
                "xt": xtn,
                "wq": np.ascontiguousarray(Wqkv[:, qcols]).astype(np_dt(qkv_dt)),
                "wk": np.ascontiguousarray(Wqkv[:, kcols]).astype(np_dt(qkv_dt)),
                "wv": np.ascontiguousarray(Wqkv[:, vcols]).astype(np_dt(qkv_dt)),
                "wp": np.ascontiguousarray(
                    Wproj[wp_row_perm, 128 * c : 128 * (c + 1)]
                ).astype(np_dt(proj_dt)),
                "bq": np.ascontiguousarray(bqkv[qcols]).reshape(128, 1),
                "bk": np.ascontiguousarray(bqkv[kcols]).reshape(128, 1),
                "bv": np.ascontiguousarray(bqkv[vcols]).reshape(128, 1),
                "bp": np.ascontiguousarray(
                    bproj[128 * c : 128 * (c + 1)]
                ).reshape(128, 1),
                "ident": identity,
            }
        )
    return in_maps


def assemble(results):
    outT = np.concatenate([r["o"] for r in results], axis=0)  # [D, TOK]
    return np.ascontiguousarray(outT.T).reshape(B, N, D).astype(np.float32)


# Chosen dtype config (see build_nc docstring for the tradeoff).
CONFIG = {"qkv_dt": BF16, "attn_dt": F16, "proj_dt": BF16}

_NC_CACHE = {}


def get_nc():
    if "nc" not in _NC_CACHE:
        _NC_CACHE["nc"] = build_nc(**CONFIG)
    return _NC_CACHE["nc"]


def kernel(x, Wqkv, bqkv, Wproj, bproj):
    nc = get_nc()
    in_maps = prep_in_maps(x, Wqkv, bqkv, Wproj, bproj, **CONFIG)
    res = run_bass_kernel_spmd(nc, in_maps, list(range(NCORES)))
    return assemble(res.results)
